# revision 7
# baseline (speedup 1.0000x reference)
"""Trainium2 Bass kernel for Gemma3 sliding-window attention.

Problem: B=1, T=4096, d_model=2048, 8 query heads / 4 KV heads, head_dim=256,
sliding window 1024, per-head RMSNorm + RoPE (interleaved rotate-half with
cat(freqs,freqs) tables), o_proj.

Sharding (8 cores): 4 KV-head groups x 2 sequence halves. Core (g, s) computes
query heads {2g, 2g+1} and KV head g for query tokens [s*2048, (s+1)*2048).
Each core projects K/V only for its OWN 2048 tokens; the 1024-token KV halo
(post RMSNorm+RoPE, bf16) is exchanged with the pair core through a
2-core HBM AllGather — every core uniformly reads back the rank-0 half
(for s=0 cores the received halo is garbage, but those j-tiles are killed
by the exp bias, keeping the program SPMD-uniform). Attention blocks run in
order a=2,3,0,1 so the halo-consuming blocks come last, hiding the
collective. Each core emits a partial o-projection [2048, 2048] in bf16;
the host sums the 4 group partials per half.

Frame indexing: j-tiles live in a 6*512-token frame [s*2048-1024, (s+1)*2048);
frame tiles 0,1 = halo (received), 2..5 = own (computed). Attention block a
covers query frame tile 2+a and j frame tiles a, a+1, a+2.

Dataflow: host pre-transposes x and all weights into partition-major
contiguous layouts so every DMA is wide. Projections and attention matmuls
run in bf16 (fp32 PSUM accumulation); rsqrt for RMSNorm is exp(-0.5*ln(x))
and the activation-table candidates are restricted so ACT stays on the
natural_log_exp table for the whole kernel (a single table load).
Attention per 512-query block in S.T orientation: S.T[j,i] = kT.T @ qT per
128-j tile, P.T = exp(S.T/16 + bias), constant triangle masks on the 8
window-edge tiles, softmax denominator via ones-matmul (no max-subtraction:
RMSNorm bounds |scores| <= 16), y.T = v.T @ P.T scaled by the reciprocal
denominator, consumed as lhsT by the o-projection.
"""

import sys

if "/opt/trn_rl_repo" not in sys.path:
    sys.path.insert(0, "/opt/trn_rl_repo")

import numpy as np

try:
    import ml_dtypes
    BF16 = ml_dtypes.bfloat16
except ImportError:
    BF16 = None

T, DM, NH, NKV, HD, WIN = 4096, 2048, 8, 4, 256, 1024
EPS, BASE = 1e-6, 10000.0
NG, NS = 4, 2
NQ = 2048
NKO = 16          # 2048 / 128 contraction subtiles
NA = 4            # 512-query attention blocks per core
NOT = 4           # own 512-token tiles per core
SCALE = 1.0 / 16.0
NEG = -1.0e5

_cache = {}


def _host_prep(x, pos, Wq, Wk, Wv, Wo, q_norm_w, k_norm_w):
    x = np.asarray(x, np.float32).reshape(T, DM)
    xT = np.ascontiguousarray(x.T)
    pos_f = np.asarray(pos).astype(np.float64)
    m = np.arange(128)
    invf = BASE ** (-m / 128.0)

    Wq = np.asarray(Wq, np.float32)
    Wk = np.asarray(Wk, np.float32)
    Wv = np.asarray(Wv, np.float32)
    Wo = np.asarray(Wo, np.float32)
    qnw = np.asarray(q_norm_w, np.float32)
    knw = np.asarray(k_norm_w, np.float32)

    ones = np.ones((128, 128), np.float32)
    r0T = np.zeros((128, 128), np.float32)
    a = np.arange(64)
    r0T[2 * a, 2 * a + 1] = 1.0
    r0T[2 * a + 1, 2 * a] = -1.0
    qw2 = np.ascontiguousarray(np.stack([qnw[:128], qnw[128:]], axis=1))
    kw2 = np.ascontiguousarray(np.stack([knw[:128], knw[128:]], axis=1))

    # masks for 512-wide attention blocks: m=0..3 far edge, m=8..11 diagonal
    jp = np.arange(128)[:, None]
    ip = np.arange(512)[None, :]
    tris = []
    for mm_ in range(4):
        tris.append(jp >= ip + 1 - 128 * mm_)         # far masks F_m
    for mm_ in range(4):
        tris.append(jp <= ip - 128 * mm_)             # diag masks D_{m+8}
    tri = np.concatenate(tris, axis=1).astype(BF16)   # [128, 8*512]

    def pack_w(WT, cols):
        # WT: [DM, cols] -> [128, NKO, cols] partition-major contiguous
        return np.ascontiguousarray(
            WT.reshape(NKO, 128, cols).transpose(1, 0, 2)).astype(BF16)

    in_maps = []
    for g in range(NG):
        for s in range(NS):
            xT_c = xT[:, s * 2048:(s + 1) * 2048]
            # [128, NOT*2, 8, 512]: index (own_tile*2 + half) -> 8KB/partition
            xt4 = np.ascontiguousarray(
                xT_c.reshape(2, 8, 128, NOT, 512)
                .transpose(2, 3, 0, 1, 4)
                .reshape(128, NOT * 2, 8, 512)).astype(BF16)
            p = pos_f[s * 2048:(s + 1) * 2048]
            ang = p[None, :] * invf[:, None]
            cosk = np.ascontiguousarray(np.cos(ang), dtype=np.float32)
            sink = np.ascontiguousarray(np.sin(ang), dtype=np.float32)

            kbias = np.zeros((128, 24), np.float32)
            if s == 0:
                kbias[:, :8] = NEG

            wqT = Wq[2 * g * HD:(2 * g + 2) * HD, :].T   # [DM, 512]
            wkT = Wk[g * HD:(g + 1) * HD, :].T           # [DM, 256]
            wvT = Wv[g * HD:(g + 1) * HD, :].T           # [DM, 256]
            woT = Wo[:, 2 * g * HD:(2 * g + 2) * HD].T   # [512, DM]
            wo4 = np.ascontiguousarray(
                woT.reshape(4, 128, DM).transpose(1, 0, 2)).astype(BF16)

            in_maps.append({
                "xt4": xt4,
                "cosk": cosk,
                "sink": sink,
                "wqT": pack_w(wqT, 512),
                "wkT": pack_w(wkT, 256),
                "wvT": pack_w(wvT, 256),
                "woT": wo4,
                "ones_bf": ones.astype(BF16),
                "r0T": r0T.astype(BF16),
                "qw": qw2,
                "kw": kw2,
                "kbias": kbias,
                "tri": tri,
            })
    return in_maps


def _build_program():
    if "nc" in _cache:
        return _cache["nc"]

    import concourse.bass as bass
    import concourse.mybir as mybir
    import concourse.tile as tile
    from concourse import bacc
    from contextlib import ExitStack

    f32 = mybir.dt.float32
    bf16 = mybir.dt.bfloat16
    AF = mybir.ActivationFunctionType
    OP = mybir.AluOpType

    nc = bacc.Bacc("TRN2", target_bir_lowering=False, debug=False,
                   enable_asserts=False, num_devices=8)

    # All ACT functions used here (Exp, Ln, Square, Copy) are members of the
    # natural_log_exp_and_others table set, but the placement pass assigns
    # each function its first matching set, which thrashes ~2.7us table
    # reloads between rmsnorm and attention. Restrict the candidates this
    # instance's pass sees (positions preserved -> set ids stay valid) so a
    # single table load serves the whole kernel.
    import types as _types
    from concourse.hw_specs import get_activation_tables as _gat
    import bass_rust as _bass_rust

    def _pinned_act_table_loads(self):
        has_activation = any(
            isinstance(i, mybir.InstActivation)
            for b in self.main_func.blocks
            for i in b.instructions
        )
        if not has_activation:
            return
        AF_ = mybir.ActivationFunctionType
        ours = {AF_.Exp, AF_.Ln, AF_.Square, AF_.Copy}
        tables = []
        for name, fns in _gat(self.m.arch).items():
            if name != "natural_log_exp_and_others":
                fns = fns - ours
            tables.append((name, fns))
        _bass_rust.insert_act_table_loads(self, tables)

    nc.insert_act_table_loads = _types.MethodType(_pinned_act_table_loads, nc)

    xt4_d = nc.dram_tensor("xt4", [128, NOT * 2, 8, 512], bf16, kind="ExternalInput")
    cosk_d = nc.dram_tensor("cosk", [128, 2048], f32, kind="ExternalInput")
    sink_d = nc.dram_tensor("sink", [128, 2048], f32, kind="ExternalInput")
    wq_d = nc.dram_tensor("wqT", [128, NKO, 512], bf16, kind="ExternalInput")
    wk_d = nc.dram_tensor("wkT", [128, NKO, 256], bf16, kind="ExternalInput")
    wv_d = nc.dram_tensor("wvT", [128, NKO, 256], bf16, kind="ExternalInput")
    wo_d = nc.dram_tensor("woT", [128, 4, DM], bf16, kind="ExternalInput")
    onesbf_d = nc.dram_tensor("ones_bf", [128, 128], bf16, kind="ExternalInput")
    r0_d = nc.dram_tensor("r0T", [128, 128], bf16, kind="ExternalInput")
    qw_d = nc.dram_tensor("qw", [128, 2], f32, kind="ExternalInput")
    kw_d = nc.dram_tensor("kw", [128, 2], f32, kind="ExternalInput")
    kb_d = nc.dram_tensor("kbias", [128, 24], f32, kind="ExternalInput")
    tri_d = nc.dram_tensor("tri", [128, 8 * 512], bf16, kind="ExternalInput")
    o_d = nc.dram_tensor("o_part", [NQ, DM], bf16, kind="ExternalOutput")

    PAIRS = [[0, 1], [2, 3], [4, 5], [6, 7]]

    with tile.TileContext(nc) as tc, ExitStack() as ctx:
        cpool = ctx.enter_context(tc.tile_pool(name="consts", bufs=1))
        xpool = ctx.enter_context(tc.tile_pool(name="xt", bufs=3))
        tabpool = ctx.enter_context(tc.tile_pool(name="tab", bufs=4))
        kpool = ctx.enter_context(tc.tile_pool(name="kring", bufs=6))
        vpool = ctx.enter_context(tc.tile_pool(name="vring", bufs=6))
        scpool = ctx.enter_context(tc.tile_pool(name="scratch", bufs=3))
        spool = ctx.enter_context(tc.tile_pool(name="small", bufs=3))
        qpool = ctx.enter_context(tc.tile_pool(name="qt", bufs=4))
        ptpool = ctx.enter_context(tc.tile_pool(name="pt", bufs=4))
        ypool = ctx.enter_context(tc.tile_pool(name="yt", bufs=3))
        opool = ctx.enter_context(tc.tile_pool(name="osb", bufs=3))
        dpool = ctx.enter_context(tc.tile_pool(name="dram", bufs=1, space="DRAM"))
        pp_proj = ctx.enter_context(tc.tile_pool(name="pproj", bufs=3, space="PSUM"))
        pp_small = ctx.enter_context(tc.tile_pool(name="psmall", bufs=2, space="PSUM"))
        pp_acc = ctx.enter_context(tc.tile_pool(name="pacc", bufs=3, space="PSUM"))

        # ---- resident constants / weights, ordered for startup latency ----
        wk_sb = cpool.tile([128, NKO, 256], bf16, tag="wk")
        nc.sync.dma_start(wk_sb[:, 0:8, :], wk_d.ap()[:, 0:8, :])
        xts = {}

        def load_x(o):
            xth = []
            for half in range(2):
                xt = xpool.tile([128, 8, 512], bf16, tag="xt")
                nc.sync.dma_start(xt[:], xt4_d.ap()[:, o * 2 + half, :, :])
                xth.append(xt)
            xts[o] = xth

        load_x(2)
        nc.sync.dma_start(wk_sb[:, 8:16, :], wk_d.ap()[:, 8:16, :])
        wv_sb = cpool.tile([128, NKO, 256], bf16, tag="wv")
        nc.sync.dma_start(wv_sb[:, 0:8, :], wv_d.ap()[:, 0:8, :])
        nc.sync.dma_start(wv_sb[:, 8:16, :], wv_d.ap()[:, 8:16, :])
        ones_sb = cpool.tile([128, 128], bf16, tag="ones")
        nc.sync.dma_start(ones_sb[:], onesbf_d.ap())
        onesbf_sb = ones_sb
        r0_sb = cpool.tile([128, 128], bf16, tag="r0")
        nc.sync.dma_start(r0_sb[:], r0_d.ap())
        qw_sb = cpool.tile([128, 2], f32, tag="qwt")
        nc.sync.dma_start(qw_sb[:], qw_d.ap())
        kw_sb = cpool.tile([128, 2], f32, tag="kwt")
        nc.sync.dma_start(kw_sb[:], kw_d.ap())
        kb_sb = cpool.tile([128, 24], f32, tag="kb")
        nc.sync.dma_start(kb_sb[:], kb_d.ap())
        eps_sb = cpool.tile([128, 1], f32, tag="eps")
        nc.vector.memset(eps_sb[:], EPS)
        zero_sb = cpool.tile([128, 1], f32, tag="zero")
        nc.vector.memset(zero_sb[:], 0.0)
        cos_tiles = {}
        sin_tiles = {}

        def load_tab(o):
            ct_ = tabpool.tile([128, 512], f32, tag="cos")
            nc.sync.dma_start(ct_[:], cosk_d.ap()[:, o * 512:(o + 1) * 512])
            st_ = tabpool.tile([128, 512], f32, tag="sin")
            nc.sync.dma_start(st_[:], sink_d.ap()[:, o * 512:(o + 1) * 512])
            cos_tiles[o] = ct_
            sin_tiles[o] = st_

        load_tab(2)
        wq_sb = cpool.tile([128, NKO, 512], bf16, tag="wq")
        nc.sync.dma_start(wq_sb[:, 0:8, :], wq_d.ap()[:, 0:8, :])
        nc.sync.dma_start(wq_sb[:, 8:16, :], wq_d.ap()[:, 8:16, :])
        # allocated now, loaded later (first used by attention/o blocks)
        tri_sb = cpool.tile([128, 8 * 512], bf16, tag="tri")
        wo_sb = cpool.tile([128, 4, DM], bf16, tag="wo")

        # frame tiles: 0,1 halo (exchanged), 2..5 own (computed)
        kt_tiles = [None] * 6
        vt_tiles = [None] * 6
        qt_tiles = [None] * 4

        def norm_rope(src_ps, w_sb, cos_t, sin_t, dst, dsti):
            """src_ps: two PSUM [128, 512] tiles (one head's 2 d-subtiles),
            transposed projection over 512 tokens. Writes RMSNorm+RoPE (bf16)
            into dst[:, dsti+u, :]."""
            z2 = scpool.tile([128, 2, 512], bf16, tag="z2")
            for u in range(2):
                nc.scalar.activation(z2[:, u, :], src_ps[u][:], AF.Square,
                                     bias=zero_sb[:])
            ssq = pp_small.tile([128, 512], f32, tag="psm")
            for u in range(2):
                nc.tensor.matmul(ssq[:], ones_sb[:], z2[:, u, :],
                                 start=(u == 0), stop=(u == 1))
            # rs = (ssq/HD + eps) ** -0.5 via ln+exp (one ACT table set)
            lt = spool.tile([128, 512], f32, tag="sq")
            nc.scalar.activation(lt[:], ssq[:], AF.Ln, bias=eps_sb[:],
                                 scale=1.0 / HD)
            rs = spool.tile([128, 512], f32, tag="rs")
            nc.scalar.activation(rs[:], lt[:], AF.Exp, bias=zero_sb[:],
                                 scale=-0.5)
            znw = scpool.tile([128, 2, 512], bf16, tag="znw")
            t1 = scpool.tile([128, 2, 512], f32, tag="t1")
            for u in range(2):
                nc.vector.scalar_tensor_tensor(
                    znw[:, u, :], src_ps[u][:], w_sb[:, u:u + 1], rs[:],
                    OP.mult, OP.mult)
                rot = pp_small.tile([128, 512], f32, tag="psm")
                nc.tensor.matmul(rot[:], r0_sb[:], znw[:, u, :], start=True, stop=True)
                nc.vector.tensor_tensor(t1[:, u, :], znw[:, u, :], cos_t, OP.mult)
                tmp = spool.tile([128, 512], f32, tag="tmp")
                nc.vector.tensor_tensor(tmp[:], rot[:], sin_t, OP.mult)
                nc.vector.tensor_tensor(dst[:, dsti + u, :], t1[:, u, :], tmp[:], OP.add)

        def proj_kv(o):
            """K/V projection + norm/rope for own tile o (frame tile 2+o)."""
            xth = xts.pop(o)
            cos_t = cos_tiles[o]
            sin_t = sin_tiles[o]

            k0_ps = pp_proj.tile([128, 512], f32, tag="pj")
            k1_ps = pp_proj.tile([128, 512], f32, tag="pj")
            k_ps = [k0_ps, k1_ps]
            for dsub in range(2):
                for ko in range(NKO):
                    nc.tensor.matmul(k_ps[dsub][:],
                                     wk_sb[:, ko, dsub * 128:(dsub + 1) * 128],
                                     xth[ko // 8][:, ko % 8, :],
                                     start=(ko == 0), stop=(ko == NKO - 1))
            kt = kpool.tile([128, 2, 512], bf16, tag="kt")
            norm_rope(k_ps, kw_sb, cos_t[:], sin_t[:], kt, 0)
            kt_tiles[2 + o] = kt

            vt = vpool.tile([128, 4, 256], bf16, tag="vt")
            for vh in range(2):
                v_ps = pp_proj.tile([128, 2, 256], f32, tag="pj")
                for ms in range(2):
                    msub = vh * 2 + ms
                    for ko in range(NKO):
                        nc.tensor.matmul(v_ps[:, ms, :],
                                         xth[ko // 8][:, ko % 8, msub * 128:(msub + 1) * 128],
                                         wv_sb[:, ko, :],
                                         start=(ko == 0), stop=(ko == NKO - 1))
                for ms in range(2):
                    nc.vector.tensor_copy(vt[:, vh * 2 + ms, :], v_ps[:, ms, :])
            vt_tiles[2 + o] = vt

            # q projections (2 heads, N=512) for query tile o
            qt_sb = qpool.tile([128, 4, 512], bf16, tag="q")
            for h in range(2):
                q0_ps = pp_proj.tile([128, 512], f32, tag="pj")
                q1_ps = pp_proj.tile([128, 512], f32, tag="pj")
                q_ps = [q0_ps, q1_ps]
                for u in range(2):
                    dsub = 2 * h + u
                    for ko in range(NKO):
                        nc.tensor.matmul(q_ps[u][:],
                                         wq_sb[:, ko, dsub * 128:(dsub + 1) * 128],
                                         xth[ko // 8][:, ko % 8, :],
                                         start=(ko == 0), stop=(ko == NKO - 1))
                norm_rope(q_ps, qw_sb, cos_t[:], sin_t[:], qt_sb, 2 * h)
            qt_tiles[o] = qt_sb

        # ---- projection phase: own tiles 2,3 first (the pair's halo), then
        # the halo exchange, then own tiles 0,1 ----
        proj_kv(2)
        load_x(3)
        load_tab(3)
        proj_kv(3)

        # halo exchange: my frame tiles 4,5 (own 2,3) -> pair's frame 0,1.
        # Both pair members contribute; everyone reads back the rank-0 half
        # (masked garbage on s=0 cores).
        send_b = dpool.tile([128, 4096], bf16, tag="sendb")
        nc.sync.dma_start(send_b[:, 0:1024],
                          kt_tiles[4][:].rearrange("p a b -> p (a b)"))
        nc.sync.dma_start(send_b[:, 1024:2048],
                          kt_tiles[5][:].rearrange("p a b -> p (a b)"))
        nc.sync.dma_start(send_b[:, 2048:3072],
                          vt_tiles[4][:].rearrange("p a b -> p (a b)"))
        nc.sync.dma_start(send_b[:, 3072:4096],
                          vt_tiles[5][:].rearrange("p a b -> p (a b)"))
        recv_b = dpool.tile([2, 128, 4096], bf16, tag="recvb")
        nc.gpsimd.collective_compute(
            "AllGather",
            mybir.AluOpType.bypass,
            replica_groups=PAIRS,
            ins=[send_b.opt()],
            outs=[recv_b.opt()],
        )
        for f in range(2):
            kt = kpool.tile([128, 2, 512], bf16, tag="kt")
            nc.sync.dma_start(kt[:].rearrange("p a b -> p (a b)"),
                              recv_b[0, :, f * 1024:(f + 1) * 1024])
            kt_tiles[f] = kt
            vt = vpool.tile([128, 4, 256], bf16, tag="vt")
            nc.sync.dma_start(vt[:].rearrange("p a b -> p (a b)"),
                              recv_b[0, :, 2048 + f * 1024:2048 + (f + 1) * 1024])
            vt_tiles[f] = vt

        load_x(0)
        load_tab(0)
        nc.sync.dma_start(tri_sb[:], tri_d.ap())
        proj_kv(0)
        load_x(1)
        load_tab(1)
        nc.sync.dma_start(wo_sb[:], wo_d.ap())
        proj_kv(1)

        # ---- attention phase: halo-free blocks first ----
        for a in (2, 3, 0, 1):
            qt_sb = qt_tiles[a]

            # attention for 512-query block a
            yt_sb = ypool.tile([128, 4, 512], bf16, tag="y")
            for h in range(2):
                dn_ps = pp_acc.tile([128, 512], f32, tag="pac")
                y0_ps = pp_acc.tile([128, 512], f32, tag="pac")
                y1_ps = pp_acc.tile([128, 512], f32, tag="pac")
                y_ps = [y0_ps, y1_ps]
                for mi, mrel in enumerate([3, 0, 1, 2] + list(range(4, 12))):
                    jt = 4 * a + mrel
                    ct, jh = jt // 4, jt % 4
                    ktc = kt_tiles[ct]
                    vtc = vt_tiles[ct]
                    # active query range: edge tiles are mostly masked
                    if mrel <= 2:
                        ia, ib = 0, 128 * (mrel + 1)
                    elif mrel >= 9:
                        ia, ib = 128 * (mrel - 8), 512
                    else:
                        ia, ib = 0, 512
                    pt = ptpool.tile([128, 512], bf16, tag="p")
                    st = pp_small.tile([128, 512], f32, tag="psm")
                    for u in range(2):
                        nc.tensor.matmul(st[:, ia:ib],
                                         ktc[:, u, jh * 128:(jh + 1) * 128],
                                         qt_sb[:, 2 * h + u, ia:ib],
                                         start=(u == 0), stop=(u == 1))
                    nc.scalar.activation(pt[:, ia:ib], st[:, ia:ib], AF.Exp,
                                         bias=kb_sb[:, jt:jt + 1], scale=SCALE)
                    if mrel < 4:
                        nc.vector.tensor_tensor(
                            pt[:, ia:ib], pt[:, ia:ib],
                            tri_sb[:, mrel * 512 + ia:mrel * 512 + ib], OP.mult)
                    elif mrel >= 8:
                        nc.vector.tensor_tensor(
                            pt[:, ia:ib], pt[:, ia:ib],
                            tri_sb[:, (mrel - 4) * 512 + ia:(mrel - 4) * 512 + ib],
                            OP.mult)
                    first, last = (mi == 0), (mrel == 11)
                    nc.tensor.matmul(dn_ps[:, ia:ib], onesbf_sb[:], pt[:, ia:ib],
                                     start=first, stop=last, skip_group_check=True)
                    for dh in range(2):
                        nc.tensor.matmul(y_ps[dh][:, ia:ib],
                                         vtc[:, jh, dh * 128:(dh + 1) * 128],
                                         pt[:, ia:ib], start=first, stop=last,
                                         skip_group_check=True)
                rc = spool.tile([128, 512], f32, tag="rc")
                nc.vector.reciprocal_approx_fast(rc[:], dn_ps[:])
                for dh in range(2):
                    nc.vector.tensor_tensor(yt_sb[:, 2 * h + dh, :],
                                            y_ps[dh][:], rc[:], OP.mult)

            # partial o-projection for the 512-query block
            for msub in range(4):
                for dmh in range(2):
                    o_sb = opool.tile([128, 1024], bf16, tag="o")
                    for dq in range(2):
                        c0 = (dmh * 2 + dq) * 512
                        o_ps = pp_small.tile([128, 512], f32, tag="psm")
                        for hd in range(4):
                            nc.tensor.matmul(o_ps[:],
                                             yt_sb[:, hd, msub * 128:(msub + 1) * 128],
                                             wo_sb[:, hd, c0:c0 + 512],
                                             start=(hd == 0), stop=(hd == 3))
                        if dq == 0:
                            nc.scalar.copy(o_sb[:, dq * 512:(dq + 1) * 512], o_ps[:])
                        else:
                            nc.vector.tensor_copy(o_sb[:, dq * 512:(dq + 1) * 512], o_ps[:])
                    r0_ = a * 512 + msub * 128
                    nc.sync.dma_start(o_d.ap()[r0_:r0_ + 128, dmh * 1024:(dmh + 1) * 1024],
                                      o_sb[:])

    nc.compile()
    _cache["nc"] = nc
    return nc


def _run(inputs, trace=False):
    from concourse.bass_utils import run_bass_kernel_spmd

    nc = _build_program()
    in_maps = _host_prep(**inputs)
    res = run_bass_kernel_spmd(nc, in_maps, core_ids=list(range(8)), trace=trace)
    full = np.zeros((T, DM), np.float32)
    for g in range(NG):
        for s in range(NS):
            full[s * 2048:(s + 1) * 2048] += np.asarray(
                res.results[g * 2 + s]["o_part"], dtype=np.float32)
    return full.reshape(1, T, DM), res


def kernel(**inputs):
    return _run(inputs, trace=False)[0]


# revision 8
# speedup vs baseline: 1.3559x; 1.3559x over previous
"""Trainium2 Bass kernel for Gemma3 sliding-window attention.

Problem: B=1, T=4096, d_model=2048, 8 query heads / 4 KV heads, head_dim=256,
sliding window 1024, per-head RMSNorm + RoPE (interleaved rotate-half with
cat(freqs,freqs) tables), o_proj.

Sharding (8 cores): 4 KV-head groups x 2 sequence halves. Core (g, s) computes
query heads {2g, 2g+1} and KV head g for query tokens [s*2048, (s+1)*2048).
Each core projects K/V only for its OWN 2048 tokens; the 1024-token KV halo
(post RMSNorm+RoPE, bf16) is exchanged with the pair core through a
2-core HBM AllGather — every core uniformly reads back the rank-0 half
(for s=0 cores the received halo is garbage, but those j-tiles are killed
by the exp bias, keeping the program SPMD-uniform). Attention blocks run in
order a=2,3,0,1 so the halo-consuming blocks come last, hiding the
collective. Each core emits a partial o-projection [2048, 2048] in bf16;
the host sums the 4 group partials per half.

Frame indexing: j-tiles live in a 6*512-token frame [s*2048-1024, (s+1)*2048);
frame tiles 0,1 = halo (received), 2..5 = own (computed). Attention block a
covers query frame tile 2+a and j frame tiles a, a+1, a+2.

Dataflow: host pre-transposes x and all weights into partition-major
contiguous layouts so every DMA is wide. Projections and attention matmuls
run in bf16 (fp32 PSUM accumulation); rsqrt for RMSNorm is exp(-0.5*ln(x))
and the activation-table candidates are restricted so ACT stays on the
natural_log_exp table for the whole kernel (a single table load).
Attention per 512-query block in S.T orientation: S.T[j,i] = kT.T @ qT per
128-j tile, P.T = exp(S.T/16 + bias), constant triangle masks on the 8
window-edge tiles, softmax denominator via ones-matmul (no max-subtraction:
RMSNorm bounds |scores| <= 16), y.T = v.T @ P.T scaled by the reciprocal
denominator, consumed as lhsT by the o-projection.
"""

import sys

if "/opt/trn_rl_repo" not in sys.path:
    sys.path.insert(0, "/opt/trn_rl_repo")

import numpy as np

try:
    import ml_dtypes
    BF16 = ml_dtypes.bfloat16
except ImportError:
    BF16 = None

T, DM, NH, NKV, HD, WIN = 4096, 2048, 8, 4, 256, 1024
EPS, BASE = 1e-6, 10000.0
NG, NS = 4, 2
NQ = 2048
NKO = 16          # 2048 / 128 contraction subtiles
NA = 4            # 512-query attention blocks per core
NOT = 4           # own 512-token tiles per core
SCALE = 1.0 / 16.0
NEG = -1.0e5

_cache = {}


def _host_prep(x, pos, Wq, Wk, Wv, Wo, q_norm_w, k_norm_w):
    x = np.asarray(x, np.float32).reshape(T, DM)
    xT = np.ascontiguousarray(x.T)
    pos_f = np.asarray(pos).astype(np.float64)
    m = np.arange(128)
    invf = BASE ** (-m / 128.0)

    Wq = np.asarray(Wq, np.float32)
    Wk = np.asarray(Wk, np.float32)
    Wv = np.asarray(Wv, np.float32)
    Wo = np.asarray(Wo, np.float32)
    qnw = np.asarray(q_norm_w, np.float32)
    knw = np.asarray(k_norm_w, np.float32)

    ones = np.ones((128, 128), np.float32)
    r0T = np.zeros((128, 128), np.float32)
    a = np.arange(64)
    r0T[2 * a, 2 * a + 1] = 1.0
    r0T[2 * a + 1, 2 * a] = -1.0
    qw2 = np.ascontiguousarray(np.stack([qnw[:128], qnw[128:]], axis=1))
    kw2 = np.ascontiguousarray(np.stack([knw[:128], knw[128:]], axis=1))

    # masks for 512-wide attention blocks: m=0..3 far edge, m=8..11 diagonal
    jp = np.arange(128)[:, None]
    ip = np.arange(512)[None, :]
    tris = []
    for mm_ in range(4):
        tris.append(jp >= ip + 1 - 128 * mm_)         # far masks F_m
    for mm_ in range(4):
        tris.append(jp <= ip - 128 * mm_)             # diag masks D_{m+8}
    tri = np.concatenate(tris, axis=1).astype(BF16)   # [128, 8*512]

    def pack_w(WT, cols):
        # WT: [DM, cols] -> [128, NKO, cols] partition-major contiguous
        return np.ascontiguousarray(
            WT.reshape(NKO, 128, cols).transpose(1, 0, 2)).astype(BF16)

    in_maps = []
    for g in range(NG):
        for s in range(NS):
            xT_c = xT[:, s * 2048:(s + 1) * 2048]
            # [128, NOT*2, 8, 512]: index (own_tile*2 + half) -> 8KB/partition
            xt4 = np.ascontiguousarray(
                xT_c.reshape(2, 8, 128, NOT, 512)
                .transpose(2, 3, 0, 1, 4)
                .reshape(128, NOT * 2, 8, 512)).astype(BF16)
            p = pos_f[s * 2048:(s + 1) * 2048]
            ang = p[None, :] * invf[:, None]
            cosk = np.ascontiguousarray(np.cos(ang), dtype=np.float32)
            sink = np.ascontiguousarray(np.sin(ang), dtype=np.float32)

            kbias = np.zeros((128, 24), np.float32)
            if s == 0:
                kbias[:, :8] = NEG

            wqT = Wq[2 * g * HD:(2 * g + 2) * HD, :].T   # [DM, 512]
            wkT = Wk[g * HD:(g + 1) * HD, :].T           # [DM, 256]
            wvT = Wv[g * HD:(g + 1) * HD, :].T           # [DM, 256]
            woT = Wo[:, 2 * g * HD:(2 * g + 2) * HD].T   # [512, DM]
            wo4 = np.ascontiguousarray(
                woT.reshape(4, 128, DM).transpose(1, 0, 2)).astype(BF16)

            in_maps.append({
                "xt4": xt4,
                "cosk": cosk,
                "sink": sink,
                "wqT": pack_w(wqT, 512),
                "wkT": pack_w(wkT, 256),
                "wvT": pack_w(wvT, 256),
                "woT": wo4,
                "ones_bf": ones.astype(BF16),
                "r0T": r0T.astype(BF16),
                "qw": qw2,
                "kw": kw2,
                "kbias": kbias,
                "tri": tri,
            })
    return in_maps


def _build_program():
    if "nc" in _cache:
        return _cache["nc"]

    import concourse.bass as bass
    import concourse.mybir as mybir
    import concourse.tile as tile
    from concourse import bacc
    from contextlib import ExitStack

    f32 = mybir.dt.float32
    bf16 = mybir.dt.bfloat16
    AF = mybir.ActivationFunctionType
    OP = mybir.AluOpType

    nc = bacc.Bacc("TRN2", target_bir_lowering=False, debug=False,
                   enable_asserts=False, num_devices=8)

    # All ACT functions used here (Exp, Ln, Square, Copy) are members of the
    # natural_log_exp_and_others table set, but the placement pass assigns
    # each function its first matching set, which thrashes ~2.7us table
    # reloads between rmsnorm and attention. Restrict the candidates this
    # instance's pass sees (positions preserved -> set ids stay valid) so a
    # single table load serves the whole kernel.
    import types as _types
    from concourse.hw_specs import get_activation_tables as _gat
    import bass_rust as _bass_rust

    def _pinned_act_table_loads(self):
        has_activation = any(
            isinstance(i, mybir.InstActivation)
            for b in self.main_func.blocks
            for i in b.instructions
        )
        if not has_activation:
            return
        AF_ = mybir.ActivationFunctionType
        ours = {AF_.Exp, AF_.Ln, AF_.Square, AF_.Copy}
        tables = []
        for name, fns in _gat(self.m.arch).items():
            if name != "natural_log_exp_and_others":
                fns = fns - ours
            tables.append((name, fns))
        _bass_rust.insert_act_table_loads(self, tables)

    nc.insert_act_table_loads = _types.MethodType(_pinned_act_table_loads, nc)

    xt4_d = nc.dram_tensor("xt4", [128, NOT * 2, 8, 512], bf16, kind="ExternalInput")
    cosk_d = nc.dram_tensor("cosk", [128, 2048], f32, kind="ExternalInput")
    sink_d = nc.dram_tensor("sink", [128, 2048], f32, kind="ExternalInput")
    wq_d = nc.dram_tensor("wqT", [128, NKO, 512], bf16, kind="ExternalInput")
    wk_d = nc.dram_tensor("wkT", [128, NKO, 256], bf16, kind="ExternalInput")
    wv_d = nc.dram_tensor("wvT", [128, NKO, 256], bf16, kind="ExternalInput")
    wo_d = nc.dram_tensor("woT", [128, 4, DM], bf16, kind="ExternalInput")
    onesbf_d = nc.dram_tensor("ones_bf", [128, 128], bf16, kind="ExternalInput")
    r0_d = nc.dram_tensor("r0T", [128, 128], bf16, kind="ExternalInput")
    qw_d = nc.dram_tensor("qw", [128, 2], f32, kind="ExternalInput")
    kw_d = nc.dram_tensor("kw", [128, 2], f32, kind="ExternalInput")
    kb_d = nc.dram_tensor("kbias", [128, 24], f32, kind="ExternalInput")
    tri_d = nc.dram_tensor("tri", [128, 8 * 512], bf16, kind="ExternalInput")
    o_d = nc.dram_tensor("o_part", [NQ, DM], bf16, kind="ExternalOutput")

    PAIRS = [[0, 1], [2, 3], [4, 5], [6, 7]]

    with tile.TileContext(nc) as tc, ExitStack() as ctx:
        cpool = ctx.enter_context(tc.tile_pool(name="consts", bufs=1))
        xpool = ctx.enter_context(tc.tile_pool(name="xt", bufs=3))
        tabpool = ctx.enter_context(tc.tile_pool(name="tab", bufs=4))
        kpool = ctx.enter_context(tc.tile_pool(name="kring", bufs=6))
        vpool = ctx.enter_context(tc.tile_pool(name="vring", bufs=6))
        scpool = ctx.enter_context(tc.tile_pool(name="scratch", bufs=3))
        spool = ctx.enter_context(tc.tile_pool(name="small", bufs=3))
        qpool = ctx.enter_context(tc.tile_pool(name="qt", bufs=4))
        ptpool = ctx.enter_context(tc.tile_pool(name="pt", bufs=4))
        ypool = ctx.enter_context(tc.tile_pool(name="yt", bufs=3))
        opool = ctx.enter_context(tc.tile_pool(name="osb", bufs=3))
        dpool = ctx.enter_context(tc.tile_pool(name="dram", bufs=1, space="DRAM"))
        pp_proj = ctx.enter_context(tc.tile_pool(name="pproj", bufs=3, space="PSUM"))
        pp_small = ctx.enter_context(tc.tile_pool(name="psmall", bufs=2, space="PSUM"))
        pp_acc = ctx.enter_context(tc.tile_pool(name="pacc", bufs=3, space="PSUM"))

        # ---- resident constants / weights, ordered for startup latency ----
        wk_sb = cpool.tile([128, NKO, 256], bf16, tag="wk")
        nc.sync.dma_start(wk_sb[:, 0:8, :], wk_d.ap()[:, 0:8, :])
        xts = {}

        def load_x(o):
            xth = []
            for half in range(2):
                xt = xpool.tile([128, 8, 512], bf16, tag="xt")
                nc.sync.dma_start(xt[:], xt4_d.ap()[:, o * 2 + half, :, :])
                xth.append(xt)
            xts[o] = xth

        load_x(2)
        nc.sync.dma_start(wk_sb[:, 8:16, :], wk_d.ap()[:, 8:16, :])
        wv_sb = cpool.tile([128, NKO, 256], bf16, tag="wv")
        nc.sync.dma_start(wv_sb[:, 0:8, :], wv_d.ap()[:, 0:8, :])
        nc.sync.dma_start(wv_sb[:, 8:16, :], wv_d.ap()[:, 8:16, :])
        ones_sb = cpool.tile([128, 128], bf16, tag="ones")
        nc.sync.dma_start(ones_sb[:], onesbf_d.ap())
        onesbf_sb = ones_sb
        r0_sb = cpool.tile([128, 128], bf16, tag="r0")
        nc.sync.dma_start(r0_sb[:], r0_d.ap())
        qw_sb = cpool.tile([128, 2], f32, tag="qwt")
        nc.sync.dma_start(qw_sb[:], qw_d.ap())
        kw_sb = cpool.tile([128, 2], f32, tag="kwt")
        nc.sync.dma_start(kw_sb[:], kw_d.ap())
        kb_sb = cpool.tile([128, 24], f32, tag="kb")
        nc.sync.dma_start(kb_sb[:], kb_d.ap())
        eps_sb = cpool.tile([128, 1], f32, tag="eps")
        nc.vector.memset(eps_sb[:], EPS)
        zero_sb = cpool.tile([128, 1], f32, tag="zero")
        nc.vector.memset(zero_sb[:], 0.0)
        cos_tiles = {}
        sin_tiles = {}

        def load_tab(o):
            ct_ = tabpool.tile([128, 512], f32, tag="cos")
            nc.sync.dma_start(ct_[:], cosk_d.ap()[:, o * 512:(o + 1) * 512])
            st_ = tabpool.tile([128, 512], f32, tag="sin")
            nc.sync.dma_start(st_[:], sink_d.ap()[:, o * 512:(o + 1) * 512])
            cos_tiles[o] = ct_
            sin_tiles[o] = st_

        load_tab(2)
        wq_sb = cpool.tile([128, NKO, 512], bf16, tag="wq")
        nc.sync.dma_start(wq_sb[:, 0:8, :], wq_d.ap()[:, 0:8, :])
        nc.sync.dma_start(wq_sb[:, 8:16, :], wq_d.ap()[:, 8:16, :])
        # allocated now, loaded later (first used by attention/o blocks)
        tri_sb = cpool.tile([128, 8 * 512], bf16, tag="tri")
        wo_sb = cpool.tile([128, 4, DM], bf16, tag="wo")

        # frame tiles: 0,1 halo (exchanged), 2..5 own (computed)
        kt_tiles = [None] * 6
        vt_tiles = [None] * 6
        qt_tiles = [None] * 4

        def norm_rope(src_ps, w_sb, cos_t, sin_t, dst, dsti):
            """src_ps: two PSUM [128, 512] tiles (one head's 2 d-subtiles),
            transposed projection over 512 tokens. Writes RMSNorm+RoPE (bf16)
            into dst[:, dsti+u, :]."""
            z2 = scpool.tile([128, 2, 512], bf16, tag="z2")
            for u in range(2):
                nc.scalar.activation(z2[:, u, :], src_ps[u][:], AF.Square,
                                     bias=zero_sb[:])
            ssq = pp_small.tile([128, 512], f32, tag="psm")
            for u in range(2):
                nc.tensor.matmul(ssq[:], ones_sb[:], z2[:, u, :],
                                 start=(u == 0), stop=(u == 1))
            # rs = (ssq/HD + eps) ** -0.5 via ln+exp (one ACT table set)
            lt = spool.tile([128, 512], f32, tag="sq")
            nc.scalar.activation(lt[:], ssq[:], AF.Ln, bias=eps_sb[:],
                                 scale=1.0 / HD)
            rs = spool.tile([128, 512], f32, tag="rs")
            nc.scalar.activation(rs[:], lt[:], AF.Exp, bias=zero_sb[:],
                                 scale=-0.5)
            znw = scpool.tile([128, 2, 512], bf16, tag="znw")
            t1 = scpool.tile([128, 2, 512], f32, tag="t1")
            for u in range(2):
                nc.vector.scalar_tensor_tensor(
                    znw[:, u, :], src_ps[u][:], w_sb[:, u:u + 1], rs[:],
                    OP.mult, OP.mult)
                rot = pp_small.tile([128, 512], f32, tag="psm")
                nc.tensor.matmul(rot[:], r0_sb[:], znw[:, u, :], start=True, stop=True)
                nc.vector.tensor_tensor(t1[:, u, :], znw[:, u, :], cos_t, OP.mult)
                tmp = spool.tile([128, 512], f32, tag="tmp")
                nc.vector.tensor_tensor(tmp[:], rot[:], sin_t, OP.mult)
                nc.vector.tensor_tensor(dst[:, dsti + u, :], t1[:, u, :], tmp[:], OP.add)

        def proj_kv(o):
            """K/V projection + norm/rope for own tile o (frame tile 2+o)."""
            xth = xts.pop(o)
            cos_t = cos_tiles[o]
            sin_t = sin_tiles[o]

            k0_ps = pp_proj.tile([128, 512], f32, tag="pj")
            k1_ps = pp_proj.tile([128, 512], f32, tag="pj")
            k_ps = [k0_ps, k1_ps]
            for dsub in range(2):
                for ko in range(NKO):
                    nc.tensor.matmul(k_ps[dsub][:],
                                     wk_sb[:, ko, dsub * 128:(dsub + 1) * 128],
                                     xth[ko // 8][:, ko % 8, :],
                                     start=(ko == 0), stop=(ko == NKO - 1))
            kt = kpool.tile([128, 2, 512], bf16, tag="kt")
            norm_rope(k_ps, kw_sb, cos_t[:], sin_t[:], kt, 0)
            kt_tiles[2 + o] = kt

            vt = vpool.tile([128, 4, 256], bf16, tag="vt")
            for vh in range(2):
                v_ps = pp_proj.tile([128, 2, 256], f32, tag="pj")
                for ms in range(2):
                    msub = vh * 2 + ms
                    for ko in range(NKO):
                        nc.tensor.matmul(v_ps[:, ms, :],
                                         xth[ko // 8][:, ko % 8, msub * 128:(msub + 1) * 128],
                                         wv_sb[:, ko, :],
                                         start=(ko == 0), stop=(ko == NKO - 1))
                for ms in range(2):
                    nc.vector.tensor_copy(vt[:, vh * 2 + ms, :], v_ps[:, ms, :])
            vt_tiles[2 + o] = vt

            # q projections (2 heads, N=512) for query tile o
            qt_sb = qpool.tile([128, 4, 512], bf16, tag="q")
            for h in range(2):
                q0_ps = pp_proj.tile([128, 512], f32, tag="pj")
                q1_ps = pp_proj.tile([128, 512], f32, tag="pj")
                q_ps = [q0_ps, q1_ps]
                for u in range(2):
                    dsub = 2 * h + u
                    for ko in range(NKO):
                        nc.tensor.matmul(q_ps[u][:],
                                         wq_sb[:, ko, dsub * 128:(dsub + 1) * 128],
                                         xth[ko // 8][:, ko % 8, :],
                                         start=(ko == 0), stop=(ko == NKO - 1))
                norm_rope(q_ps, qw_sb, cos_t[:], sin_t[:], qt_sb, 2 * h)
            qt_tiles[o] = qt_sb

        # ---- projection phase: own tiles 2,3 first (the pair's halo), then
        # the halo exchange, then own tiles 0,1 ----
        proj_kv(2)
        load_x(3)
        load_tab(3)
        proj_kv(3)

        # halo exchange: my frame tiles 4,5 (own 2,3) -> pair's frame 0,1.
        # Both pair members contribute; everyone reads back the rank-0 half
        # (masked garbage on s=0 cores).
        # The whole exchange lives on the otherwise-idle gpsimd queue so its
        # waits never head-of-line-block the sync queue's DMA stream.
        send_b = dpool.tile([128, 4096], bf16, tag="sendb")
        nc.gpsimd.dma_start(send_b[:, 0:1024],
                            kt_tiles[4][:].rearrange("p a b -> p (a b)"))
        nc.gpsimd.dma_start(send_b[:, 1024:2048],
                            kt_tiles[5][:].rearrange("p a b -> p (a b)"))
        nc.gpsimd.dma_start(send_b[:, 2048:3072],
                            vt_tiles[4][:].rearrange("p a b -> p (a b)"))
        nc.gpsimd.dma_start(send_b[:, 3072:4096],
                            vt_tiles[5][:].rearrange("p a b -> p (a b)"))
        recv_b = dpool.tile([2, 128, 4096], bf16, tag="recvb")
        nc.gpsimd.collective_compute(
            "AllGather",
            mybir.AluOpType.bypass,
            replica_groups=PAIRS,
            ins=[send_b.opt()],
            outs=[recv_b.opt()],
        )

        load_x(0)
        load_tab(0)
        nc.sync.dma_start(tri_sb[:], tri_d.ap())
        proj_kv(0)
        load_x(1)
        load_tab(1)
        nc.sync.dma_start(wo_sb[:], wo_d.ap())
        proj_kv(1)

        def recv_halo():
            for f in range(2):
                kt = kpool.tile([128, 2, 512], bf16, tag="kt")
                nc.gpsimd.dma_start(kt[:].rearrange("p a b -> p (a b)"),
                                    recv_b[0, :, f * 1024:(f + 1) * 1024])
                kt_tiles[f] = kt
                vt = vpool.tile([128, 4, 256], bf16, tag="vt")
                nc.gpsimd.dma_start(vt[:].rearrange("p a b -> p (a b)"),
                                    recv_b[0, :, 2048 + f * 1024:2048 + (f + 1) * 1024])
                vt_tiles[f] = vt

        # ---- attention phase: halo-free blocks first ----
        for a in (2, 3, 0, 1):
            if a == 0:
                recv_halo()
            qt_sb = qt_tiles[a]

            # attention for 512-query block a
            yt_sb = ypool.tile([128, 4, 512], bf16, tag="y")
            for h in range(2):
                dn_ps = pp_acc.tile([128, 512], f32, tag="pac")
                y0_ps = pp_acc.tile([128, 512], f32, tag="pac")
                y1_ps = pp_acc.tile([128, 512], f32, tag="pac")
                y_ps = [y0_ps, y1_ps]
                for mi, mrel in enumerate([3, 0, 1, 2] + list(range(4, 12))):
                    jt = 4 * a + mrel
                    ct, jh = jt // 4, jt % 4
                    ktc = kt_tiles[ct]
                    vtc = vt_tiles[ct]
                    # active query range: edge tiles are mostly masked
                    if mrel <= 2:
                        ia, ib = 0, 128 * (mrel + 1)
                    elif mrel >= 9:
                        ia, ib = 128 * (mrel - 8), 512
                    else:
                        ia, ib = 0, 512
                    pt = ptpool.tile([128, 512], bf16, tag="p")
                    st = pp_small.tile([128, 512], f32, tag="psm")
                    for u in range(2):
                        nc.tensor.matmul(st[:, ia:ib],
                                         ktc[:, u, jh * 128:(jh + 1) * 128],
                                         qt_sb[:, 2 * h + u, ia:ib],
                                         start=(u == 0), stop=(u == 1))
                    nc.scalar.activation(pt[:, ia:ib], st[:, ia:ib], AF.Exp,
                                         bias=kb_sb[:, jt:jt + 1], scale=SCALE)
                    if mrel < 4:
                        nc.vector.tensor_tensor(
                            pt[:, ia:ib], pt[:, ia:ib],
                            tri_sb[:, mrel * 512 + ia:mrel * 512 + ib], OP.mult)
                    elif mrel >= 8:
                        nc.vector.tensor_tensor(
                            pt[:, ia:ib], pt[:, ia:ib],
                            tri_sb[:, (mrel - 4) * 512 + ia:(mrel - 4) * 512 + ib],
                            OP.mult)
                    first, last = (mi == 0), (mrel == 11)
                    nc.tensor.matmul(dn_ps[:, ia:ib], onesbf_sb[:], pt[:, ia:ib],
                                     start=first, stop=last, skip_group_check=True)
                    for dh in range(2):
                        nc.tensor.matmul(y_ps[dh][:, ia:ib],
                                         vtc[:, jh, dh * 128:(dh + 1) * 128],
                                         pt[:, ia:ib], start=first, stop=last,
                                         skip_group_check=True)
                rc = spool.tile([128, 512], f32, tag="rc")
                nc.vector.reciprocal_approx_fast(rc[:], dn_ps[:])
                for dh in range(2):
                    nc.vector.tensor_tensor(yt_sb[:, 2 * h + dh, :],
                                            y_ps[dh][:], rc[:], OP.mult)

            # partial o-projection for the 512-query block
            for msub in range(4):
                for dmh in range(2):
                    o_sb = opool.tile([128, 1024], bf16, tag="o")
                    for dq in range(2):
                        c0 = (dmh * 2 + dq) * 512
                        o_ps = pp_small.tile([128, 512], f32, tag="psm")
                        for hd in range(4):
                            nc.tensor.matmul(o_ps[:],
                                             yt_sb[:, hd, msub * 128:(msub + 1) * 128],
                                             wo_sb[:, hd, c0:c0 + 512],
                                             start=(hd == 0), stop=(hd == 3))
                        if dq == 0:
                            nc.scalar.copy(o_sb[:, dq * 512:(dq + 1) * 512], o_ps[:])
                        else:
                            nc.vector.tensor_copy(o_sb[:, dq * 512:(dq + 1) * 512], o_ps[:])
                    r0_ = a * 512 + msub * 128
                    nc.sync.dma_start(o_d.ap()[r0_:r0_ + 128, dmh * 1024:(dmh + 1) * 1024],
                                      o_sb[:])

    nc.compile()
    _cache["nc"] = nc
    return nc


def _run(inputs, trace=False):
    from concourse.bass_utils import run_bass_kernel_spmd

    nc = _build_program()
    in_maps = _host_prep(**inputs)
    res = run_bass_kernel_spmd(nc, in_maps, core_ids=list(range(8)), trace=trace)
    full = np.zeros((T, DM), np.float32)
    for g in range(NG):
        for s in range(NS):
            full[s * 2048:(s + 1) * 2048] += np.asarray(
                res.results[g * 2 + s]["o_part"], dtype=np.float32)
    return full.reshape(1, T, DM), res


def kernel(**inputs):
    return _run(inputs, trace=False)[0]


# revision 10
# speedup vs baseline: 1.4170x; 1.0451x over previous
"""Trainium2 Bass kernel for Gemma3 sliding-window attention.

Problem: B=1, T=4096, d_model=2048, 8 query heads / 4 KV heads, head_dim=256,
sliding window 1024, per-head RMSNorm + RoPE (interleaved rotate-half with
cat(freqs,freqs) tables), o_proj.

Sharding (8 cores): 4 KV-head groups x 2 sequence halves. Core (g, s) computes
query heads {2g, 2g+1} and KV head g for query tokens [s*2048, (s+1)*2048).
Each core projects K/V only for its OWN 2048 tokens; the 1024-token KV halo
(post RMSNorm+RoPE, bf16) is exchanged with the pair core through a
2-core HBM AllGather — every core uniformly reads back the rank-0 half
(for s=0 cores the received halo is garbage, but those j-tiles are killed
by the exp bias, keeping the program SPMD-uniform). Attention blocks run in
order a=2,3,0,1 so the halo-consuming blocks come last, hiding the
collective. Each core emits a partial o-projection [2048, 2048] in bf16;
the host sums the 4 group partials per half.

Frame indexing: j-tiles live in a 6*512-token frame [s*2048-1024, (s+1)*2048);
frame tiles 0,1 = halo (received), 2..5 = own (computed). Attention block a
covers query frame tile 2+a and j frame tiles a, a+1, a+2.

Dataflow: host pre-transposes x and all weights into partition-major
contiguous layouts so every DMA is wide. Projections and attention matmuls
run in bf16 (fp32 PSUM accumulation); rsqrt for RMSNorm is exp(-0.5*ln(x))
and the activation-table candidates are restricted so ACT stays on the
natural_log_exp table for the whole kernel (a single table load).
Attention per 512-query block in S.T orientation: S.T[j,i] = kT.T @ qT per
128-j tile, P.T = exp(S.T/16 + bias), constant triangle masks on the 8
window-edge tiles, softmax denominator via ones-matmul (no max-subtraction:
RMSNorm bounds |scores| <= 16), y.T = v.T @ P.T scaled by the reciprocal
denominator, consumed as lhsT by the o-projection.
"""

import sys

if "/opt/trn_rl_repo" not in sys.path:
    sys.path.insert(0, "/opt/trn_rl_repo")

import numpy as np

try:
    import ml_dtypes
    BF16 = ml_dtypes.bfloat16
except ImportError:
    BF16 = None

T, DM, NH, NKV, HD, WIN = 4096, 2048, 8, 4, 256, 1024
EPS, BASE = 1e-6, 10000.0
NG, NS = 4, 2
NQ = 2048
NKO = 16          # 2048 / 128 contraction subtiles
NA = 4            # 512-query attention blocks per core
NOT = 4           # own 512-token tiles per core
SCALE = 1.0 / 16.0
NEG = -1.0e5

_cache = {}


def _host_prep(x, pos, Wq, Wk, Wv, Wo, q_norm_w, k_norm_w):
    x = np.asarray(x, np.float32).reshape(T, DM)
    xT = np.ascontiguousarray(x.T)
    pos_f = np.asarray(pos).astype(np.float64)
    m = np.arange(128)
    invf = BASE ** (-m / 128.0)

    Wq = np.asarray(Wq, np.float32)
    Wk = np.asarray(Wk, np.float32)
    Wv = np.asarray(Wv, np.float32)
    Wo = np.asarray(Wo, np.float32)
    qnw = np.asarray(q_norm_w, np.float32)
    knw = np.asarray(k_norm_w, np.float32)

    ones = np.ones((128, 128), np.float32)
    r0T = np.zeros((128, 128), np.float32)
    a = np.arange(64)
    r0T[2 * a, 2 * a + 1] = 1.0
    r0T[2 * a + 1, 2 * a] = -1.0
    qw2 = np.ascontiguousarray(np.stack([qnw[:128], qnw[128:]], axis=1))
    kw2 = np.ascontiguousarray(np.stack([knw[:128], knw[128:]], axis=1))

    # masks for 512-wide attention blocks: m=0..3 far edge, m=8..11 diagonal
    jp = np.arange(128)[:, None]
    ip = np.arange(512)[None, :]
    tris = []
    for mm_ in range(4):
        tris.append(jp >= ip + 1 - 128 * mm_)         # far masks F_m
    for mm_ in range(4):
        tris.append(jp <= ip - 128 * mm_)             # diag masks D_{m+8}
    tri = np.concatenate(tris, axis=1).astype(BF16)   # [128, 8*512]

    def pack_w(WT, cols):
        # WT: [DM, cols] -> [128, NKO, cols] partition-major contiguous
        return np.ascontiguousarray(
            WT.reshape(NKO, 128, cols).transpose(1, 0, 2)).astype(BF16)

    in_maps = []
    for g in range(NG):
        for s in range(NS):
            xT_c = xT[:, s * 2048:(s + 1) * 2048]
            # [128, NOT*2, 8, 512]: index (own_tile*2 + half) -> 8KB/partition
            xt4 = np.ascontiguousarray(
                xT_c.reshape(2, 8, 128, NOT, 512)
                .transpose(2, 3, 0, 1, 4)
                .reshape(128, NOT * 2, 8, 512)).astype(BF16)
            p = pos_f[s * 2048:(s + 1) * 2048]
            ang = p[None, :] * invf[:, None]
            cosk = np.ascontiguousarray(np.cos(ang), dtype=np.float32)
            sink = np.ascontiguousarray(np.sin(ang), dtype=np.float32)

            kbias = np.zeros((128, 24), np.float32)
            if s == 0:
                kbias[:, :8] = NEG

            wqT = Wq[2 * g * HD:(2 * g + 2) * HD, :].T   # [DM, 512]
            wkT = Wk[g * HD:(g + 1) * HD, :].T           # [DM, 256]
            wvT = Wv[g * HD:(g + 1) * HD, :].T           # [DM, 256]
            woT = Wo[:, 2 * g * HD:(2 * g + 2) * HD].T   # [512, DM]
            wo4 = np.ascontiguousarray(
                woT.reshape(4, 128, DM).transpose(1, 0, 2)).astype(BF16)

            in_maps.append({
                "xt4": xt4,
                "cosk": cosk,
                "sink": sink,
                "wqT": pack_w(wqT, 512),
                "wkT": pack_w(wkT, 256),
                "wvT": pack_w(wvT, 256),
                "woT": wo4,
                "ones_bf": ones.astype(BF16),
                "r0T": r0T.astype(BF16),
                "qw": qw2,
                "kw": kw2,
                "kbias": kbias,
                "tri": tri,
            })
    return in_maps


def _build_program():
    if "nc" in _cache:
        return _cache["nc"]

    import concourse.bass as bass
    import concourse.mybir as mybir
    import concourse.tile as tile
    from concourse import bacc
    from contextlib import ExitStack

    f32 = mybir.dt.float32
    bf16 = mybir.dt.bfloat16
    AF = mybir.ActivationFunctionType
    OP = mybir.AluOpType

    nc = bacc.Bacc("TRN2", target_bir_lowering=False, debug=False,
                   enable_asserts=False, num_devices=8)

    # All ACT functions used here (Exp, Ln, Square, Copy) are members of the
    # natural_log_exp_and_others table set, but the placement pass assigns
    # each function its first matching set, which thrashes ~2.7us table
    # reloads between rmsnorm and attention. Restrict the candidates this
    # instance's pass sees (positions preserved -> set ids stay valid) so a
    # single table load serves the whole kernel.
    import types as _types
    from concourse.hw_specs import get_activation_tables as _gat
    import bass_rust as _bass_rust

    def _pinned_act_table_loads(self):
        has_activation = any(
            isinstance(i, mybir.InstActivation)
            for b in self.main_func.blocks
            for i in b.instructions
        )
        if not has_activation:
            return
        AF_ = mybir.ActivationFunctionType
        ours = {AF_.Exp, AF_.Ln, AF_.Square, AF_.Copy}
        tables = []
        for name, fns in _gat(self.m.arch).items():
            if name != "natural_log_exp_and_others":
                fns = fns - ours
            tables.append((name, fns))
        _bass_rust.insert_act_table_loads(self, tables)

    nc.insert_act_table_loads = _types.MethodType(_pinned_act_table_loads, nc)

    xt4_d = nc.dram_tensor("xt4", [128, NOT * 2, 8, 512], bf16, kind="ExternalInput")
    cosk_d = nc.dram_tensor("cosk", [128, 2048], f32, kind="ExternalInput")
    sink_d = nc.dram_tensor("sink", [128, 2048], f32, kind="ExternalInput")
    wq_d = nc.dram_tensor("wqT", [128, NKO, 512], bf16, kind="ExternalInput")
    wk_d = nc.dram_tensor("wkT", [128, NKO, 256], bf16, kind="ExternalInput")
    wv_d = nc.dram_tensor("wvT", [128, NKO, 256], bf16, kind="ExternalInput")
    wo_d = nc.dram_tensor("woT", [128, 4, DM], bf16, kind="ExternalInput")
    onesbf_d = nc.dram_tensor("ones_bf", [128, 128], bf16, kind="ExternalInput")
    r0_d = nc.dram_tensor("r0T", [128, 128], bf16, kind="ExternalInput")
    qw_d = nc.dram_tensor("qw", [128, 2], f32, kind="ExternalInput")
    kw_d = nc.dram_tensor("kw", [128, 2], f32, kind="ExternalInput")
    kb_d = nc.dram_tensor("kbias", [128, 24], f32, kind="ExternalInput")
    tri_d = nc.dram_tensor("tri", [128, 8 * 512], bf16, kind="ExternalInput")
    o_d = nc.dram_tensor("o_part", [NQ, DM], bf16, kind="ExternalOutput")

    PAIRS = [[0, 1], [2, 3], [4, 5], [6, 7]]

    with tile.TileContext(nc) as tc, ExitStack() as ctx:
        cpool = ctx.enter_context(tc.tile_pool(name="consts", bufs=1))
        xpool = ctx.enter_context(tc.tile_pool(name="xt", bufs=3))
        tabpool = ctx.enter_context(tc.tile_pool(name="tab", bufs=4))
        kpool = ctx.enter_context(tc.tile_pool(name="kring", bufs=6))
        vpool = ctx.enter_context(tc.tile_pool(name="vring", bufs=6))
        scpool = ctx.enter_context(tc.tile_pool(name="scratch", bufs=3))
        spool = ctx.enter_context(tc.tile_pool(name="small", bufs=2))
        qpool = ctx.enter_context(tc.tile_pool(name="qt", bufs=4))
        ptpool = ctx.enter_context(tc.tile_pool(name="pt", bufs=6))
        ypool = ctx.enter_context(tc.tile_pool(name="yt", bufs=3))
        opool = ctx.enter_context(tc.tile_pool(name="osb", bufs=4))
        dpool = ctx.enter_context(tc.tile_pool(name="dram", bufs=1, space="DRAM"))
        pp_proj = ctx.enter_context(tc.tile_pool(name="pproj", bufs=3, space="PSUM"))
        pp_small = ctx.enter_context(tc.tile_pool(name="psmall", bufs=2, space="PSUM"))
        pp_acc = ctx.enter_context(tc.tile_pool(name="pacc", bufs=3, space="PSUM"))

        # ---- resident constants / weights, ordered for startup latency ----
        wk_sb = cpool.tile([128, NKO, 256], bf16, tag="wk")
        nc.sync.dma_start(wk_sb[:, 0:8, :], wk_d.ap()[:, 0:8, :])
        xts = {}

        def load_x(o):
            xth = []
            for half in range(2):
                xt = xpool.tile([128, 8, 512], bf16, tag="xt")
                nc.sync.dma_start(xt[:], xt4_d.ap()[:, o * 2 + half, :, :])
                xth.append(xt)
            xts[o] = xth

        load_x(2)
        nc.sync.dma_start(wk_sb[:, 8:16, :], wk_d.ap()[:, 8:16, :])
        wv_sb = cpool.tile([128, NKO, 256], bf16, tag="wv")
        nc.sync.dma_start(wv_sb[:, 0:8, :], wv_d.ap()[:, 0:8, :])
        nc.sync.dma_start(wv_sb[:, 8:16, :], wv_d.ap()[:, 8:16, :])
        ones_sb = cpool.tile([128, 128], bf16, tag="ones")
        nc.sync.dma_start(ones_sb[:], onesbf_d.ap())
        onesbf_sb = ones_sb
        r0_sb = cpool.tile([128, 128], bf16, tag="r0")
        nc.sync.dma_start(r0_sb[:], r0_d.ap())
        qw_sb = cpool.tile([128, 2], f32, tag="qwt")
        nc.sync.dma_start(qw_sb[:], qw_d.ap())
        kw_sb = cpool.tile([128, 2], f32, tag="kwt")
        nc.sync.dma_start(kw_sb[:], kw_d.ap())
        kb_sb = cpool.tile([128, 24], f32, tag="kb")
        nc.sync.dma_start(kb_sb[:], kb_d.ap())
        eps_sb = cpool.tile([128, 1], f32, tag="eps")
        nc.vector.memset(eps_sb[:], EPS)
        zero_sb = cpool.tile([128, 1], f32, tag="zero")
        nc.vector.memset(zero_sb[:], 0.0)
        cos_tiles = {}
        sin_tiles = {}

        def load_tab(o):
            ct_ = tabpool.tile([128, 512], f32, tag="cos")
            nc.sync.dma_start(ct_[:], cosk_d.ap()[:, o * 512:(o + 1) * 512])
            st_ = tabpool.tile([128, 512], f32, tag="sin")
            nc.sync.dma_start(st_[:], sink_d.ap()[:, o * 512:(o + 1) * 512])
            cos_tiles[o] = ct_
            sin_tiles[o] = st_

        load_tab(2)
        wq_sb = cpool.tile([128, NKO, 512], bf16, tag="wq")
        nc.sync.dma_start(wq_sb[:, 0:8, :], wq_d.ap()[:, 0:8, :])
        nc.sync.dma_start(wq_sb[:, 8:16, :], wq_d.ap()[:, 8:16, :])
        # allocated now, loaded later (first used by attention/o blocks)
        tri_sb = cpool.tile([128, 8 * 512], bf16, tag="tri")
        wo_sb = cpool.tile([128, 4, DM], bf16, tag="wo")

        # frame tiles: 0,1 halo (exchanged), 2..5 own (computed)
        kt_tiles = [None] * 6
        vt_tiles = [None] * 6
        qt_tiles = [None] * 4

        def norm_rope(src_ps, w_sb, cos_t, sin_t, dst, dsti):
            """src_ps: two PSUM [128, 512] tiles (one head's 2 d-subtiles),
            transposed projection over 512 tokens. Writes RMSNorm+RoPE (bf16)
            into dst[:, dsti+u, :]."""
            z2 = scpool.tile([128, 2, 512], bf16, tag="z2")
            for u in range(2):
                nc.scalar.activation(z2[:, u, :], src_ps[u][:], AF.Square,
                                     bias=zero_sb[:])
            ssq = pp_small.tile([128, 512], f32, tag="psm")
            for u in range(2):
                nc.tensor.matmul(ssq[:], ones_sb[:], z2[:, u, :],
                                 start=(u == 0), stop=(u == 1))
            # rs = (ssq/HD + eps) ** -0.5 via ln+exp (one ACT table set)
            lt = spool.tile([128, 512], f32, tag="sq")
            nc.scalar.activation(lt[:], ssq[:], AF.Ln, bias=eps_sb[:],
                                 scale=1.0 / HD)
            rs = spool.tile([128, 512], f32, tag="rs")
            nc.scalar.activation(rs[:], lt[:], AF.Exp, bias=zero_sb[:],
                                 scale=-0.5)
            znw = scpool.tile([128, 2, 512], bf16, tag="znw")
            t1 = scpool.tile([128, 2, 512], f32, tag="t1")
            for u in range(2):
                nc.vector.scalar_tensor_tensor(
                    znw[:, u, :], src_ps[u][:], w_sb[:, u:u + 1], rs[:],
                    OP.mult, OP.mult)
                rot = pp_small.tile([128, 512], f32, tag="psm")
                nc.tensor.matmul(rot[:], r0_sb[:], znw[:, u, :], start=True, stop=True)
                nc.vector.tensor_tensor(t1[:, u, :], znw[:, u, :], cos_t, OP.mult)
                tmp = spool.tile([128, 512], f32, tag="tmp")
                nc.vector.tensor_tensor(tmp[:], rot[:], sin_t, OP.mult)
                nc.vector.tensor_tensor(dst[:, dsti + u, :], t1[:, u, :], tmp[:], OP.add)

        def proj_kv(o):
            """K/V projection + norm/rope for own tile o (frame tile 2+o)."""
            xth = xts.pop(o)
            cos_t = cos_tiles[o]
            sin_t = sin_tiles[o]

            k0_ps = pp_proj.tile([128, 512], f32, tag="pj")
            k1_ps = pp_proj.tile([128, 512], f32, tag="pj")
            k_ps = [k0_ps, k1_ps]
            for dsub in range(2):
                for ko in range(NKO):
                    nc.tensor.matmul(k_ps[dsub][:],
                                     wk_sb[:, ko, dsub * 128:(dsub + 1) * 128],
                                     xth[ko // 8][:, ko % 8, :],
                                     start=(ko == 0), stop=(ko == NKO - 1))
            kt = kpool.tile([128, 2, 512], bf16, tag="kt")
            norm_rope(k_ps, kw_sb, cos_t[:], sin_t[:], kt, 0)
            kt_tiles[2 + o] = kt

            vt = vpool.tile([128, 4, 256], bf16, tag="vt")
            for vh in range(2):
                v_ps = pp_proj.tile([128, 2, 256], f32, tag="pj")
                for ms in range(2):
                    msub = vh * 2 + ms
                    for ko in range(NKO):
                        nc.tensor.matmul(v_ps[:, ms, :],
                                         xth[ko // 8][:, ko % 8, msub * 128:(msub + 1) * 128],
                                         wv_sb[:, ko, :],
                                         start=(ko == 0), stop=(ko == NKO - 1))
                for ms in range(2):
                    nc.vector.tensor_copy(vt[:, vh * 2 + ms, :], v_ps[:, ms, :])
            vt_tiles[2 + o] = vt

            # q projections (2 heads, N=512) for query tile o
            qt_sb = qpool.tile([128, 4, 512], bf16, tag="q")
            for h in range(2):
                q0_ps = pp_proj.tile([128, 512], f32, tag="pj")
                q1_ps = pp_proj.tile([128, 512], f32, tag="pj")
                q_ps = [q0_ps, q1_ps]
                for u in range(2):
                    dsub = 2 * h + u
                    for ko in range(NKO):
                        nc.tensor.matmul(q_ps[u][:],
                                         wq_sb[:, ko, dsub * 128:(dsub + 1) * 128],
                                         xth[ko // 8][:, ko % 8, :],
                                         start=(ko == 0), stop=(ko == NKO - 1))
                norm_rope(q_ps, qw_sb, cos_t[:], sin_t[:], qt_sb, 2 * h)
            qt_tiles[o] = qt_sb

        # ---- projection phase: own tiles 2,3 first (the pair's halo), then
        # the halo exchange, then own tiles 0,1 ----
        proj_kv(2)
        load_x(3)
        load_tab(3)
        proj_kv(3)

        # halo exchange: my frame tiles 4,5 (own 2,3) -> pair's frame 0,1.
        # Both pair members contribute; everyone reads back the rank-0 half
        # (masked garbage on s=0 cores).
        # The whole exchange lives on the otherwise-idle gpsimd queue so its
        # waits never head-of-line-block the sync queue's DMA stream.
        send_b = dpool.tile([128, 4096], bf16, tag="sendb")
        nc.gpsimd.dma_start(send_b[:, 0:1024],
                            kt_tiles[4][:].rearrange("p a b -> p (a b)"))
        nc.gpsimd.dma_start(send_b[:, 1024:2048],
                            kt_tiles[5][:].rearrange("p a b -> p (a b)"))
        nc.gpsimd.dma_start(send_b[:, 2048:3072],
                            vt_tiles[4][:].rearrange("p a b -> p (a b)"))
        nc.gpsimd.dma_start(send_b[:, 3072:4096],
                            vt_tiles[5][:].rearrange("p a b -> p (a b)"))
        recv_b = dpool.tile([2, 128, 4096], bf16, tag="recvb")
        nc.gpsimd.collective_compute(
            "AllGather",
            mybir.AluOpType.bypass,
            replica_groups=PAIRS,
            ins=[send_b.opt()],
            outs=[recv_b.opt()],
        )

        load_x(0)
        load_tab(0)
        nc.sync.dma_start(tri_sb[:], tri_d.ap())
        proj_kv(0)
        load_x(1)
        load_tab(1)
        nc.sync.dma_start(wo_sb[:], wo_d.ap())
        proj_kv(1)

        def recv_halo():
            for f in range(2):
                kt = kpool.tile([128, 2, 512], bf16, tag="kt")
                nc.gpsimd.dma_start(kt[:].rearrange("p a b -> p (a b)"),
                                    recv_b[0, :, f * 1024:(f + 1) * 1024])
                kt_tiles[f] = kt
                vt = vpool.tile([128, 4, 256], bf16, tag="vt")
                nc.gpsimd.dma_start(vt[:].rearrange("p a b -> p (a b)"),
                                    recv_b[0, :, 2048 + f * 1024:2048 + (f + 1) * 1024])
                vt_tiles[f] = vt

        # ---- attention phase: halo-free blocks first ----
        for a in (2, 3, 0, 1):
            if a == 0:
                recv_halo()
            qt_sb = qt_tiles[a]

            # attention for 512-query block a
            yt_sb = ypool.tile([128, 4, 512], bf16, tag="y")
            for h in range(2):
                dn_ps = pp_acc.tile([128, 512], f32, tag="pac")
                y0_ps = pp_acc.tile([128, 512], f32, tag="pac")
                y1_ps = pp_acc.tile([128, 512], f32, tag="pac")
                y_ps = [y0_ps, y1_ps]
                for mi, mrel in enumerate([3, 0, 1, 2] + list(range(4, 12))):
                    jt = 4 * a + mrel
                    ct, jh = jt // 4, jt % 4
                    ktc = kt_tiles[ct]
                    vtc = vt_tiles[ct]
                    # active query range: edge tiles are mostly masked
                    if mrel <= 2:
                        ia, ib = 0, 128 * (mrel + 1)
                    elif mrel >= 9:
                        ia, ib = 128 * (mrel - 8), 512
                    else:
                        ia, ib = 0, 512
                    pt = ptpool.tile([128, 512], bf16, tag="p")
                    st = pp_small.tile([128, 512], f32, tag="psm")
                    for u in range(2):
                        nc.tensor.matmul(st[:, ia:ib],
                                         ktc[:, u, jh * 128:(jh + 1) * 128],
                                         qt_sb[:, 2 * h + u, ia:ib],
                                         start=(u == 0), stop=(u == 1))
                    nc.scalar.activation(pt[:, ia:ib], st[:, ia:ib], AF.Exp,
                                         bias=kb_sb[:, jt:jt + 1], scale=SCALE)
                    if mrel < 4:
                        nc.vector.tensor_tensor(
                            pt[:, ia:ib], pt[:, ia:ib],
                            tri_sb[:, mrel * 512 + ia:mrel * 512 + ib], OP.mult)
                    elif mrel >= 8:
                        nc.vector.tensor_tensor(
                            pt[:, ia:ib], pt[:, ia:ib],
                            tri_sb[:, (mrel - 4) * 512 + ia:(mrel - 4) * 512 + ib],
                            OP.mult)
                    first, last = (mi == 0), (mrel == 11)
                    nc.tensor.matmul(dn_ps[:, ia:ib], onesbf_sb[:], pt[:, ia:ib],
                                     start=first, stop=last, skip_group_check=True)
                    for dh in range(2):
                        nc.tensor.matmul(y_ps[dh][:, ia:ib],
                                         vtc[:, jh, dh * 128:(dh + 1) * 128],
                                         pt[:, ia:ib], start=first, stop=last,
                                         skip_group_check=True)
                rc = spool.tile([128, 512], f32, tag="rc")
                nc.vector.reciprocal_approx_fast(rc[:], dn_ps[:])
                for dh in range(2):
                    nc.vector.tensor_tensor(yt_sb[:, 2 * h + dh, :],
                                            y_ps[dh][:], rc[:], OP.mult)

            # partial o-projection for the 512-query block
            for msub in range(4):
                for dmh in range(2):
                    o_sb = opool.tile([128, 1024], bf16, tag="o")
                    for dq in range(2):
                        c0 = (dmh * 2 + dq) * 512
                        o_ps = pp_proj.tile([128, 512], f32, tag="pj")
                        for hd in range(4):
                            nc.tensor.matmul(o_ps[:],
                                             yt_sb[:, hd, msub * 128:(msub + 1) * 128],
                                             wo_sb[:, hd, c0:c0 + 512],
                                             start=(hd == 0), stop=(hd == 3))
                        if dq == 0:
                            nc.scalar.copy(o_sb[:, dq * 512:(dq + 1) * 512], o_ps[:])
                        else:
                            nc.vector.tensor_copy(o_sb[:, dq * 512:(dq + 1) * 512], o_ps[:])
                    r0_ = a * 512 + msub * 128
                    nc.sync.dma_start(o_d.ap()[r0_:r0_ + 128, dmh * 1024:(dmh + 1) * 1024],
                                      o_sb[:])

    nc.compile()
    _cache["nc"] = nc
    return nc


def _run(inputs, trace=False):
    from concourse.bass_utils import run_bass_kernel_spmd

    nc = _build_program()
    in_maps = _host_prep(**inputs)
    res = run_bass_kernel_spmd(nc, in_maps, core_ids=list(range(8)), trace=trace)
    full = np.zeros((T, DM), np.float32)
    for g in range(NG):
        for s in range(NS):
            full[s * 2048:(s + 1) * 2048] += np.asarray(
                res.results[g * 2 + s]["o_part"], dtype=np.float32)
    return full.reshape(1, T, DM), res


def kernel(**inputs):
    return _run(inputs, trace=False)[0]


# revision 12
# speedup vs baseline: 1.4735x; 1.0398x over previous
"""Trainium2 Bass kernel for Gemma3 sliding-window attention.

Problem: B=1, T=4096, d_model=2048, 8 query heads / 4 KV heads, head_dim=256,
sliding window 1024, per-head RMSNorm + RoPE (interleaved rotate-half with
cat(freqs,freqs) tables), o_proj.

Sharding (8 cores): 4 KV-head groups x 2 sequence halves. Core (g, s) computes
query heads {2g, 2g+1} and KV head g for query tokens [s*2048, (s+1)*2048).
Each core projects K/V only for its OWN 2048 tokens; the 1024-token KV halo
(post RMSNorm+RoPE, bf16) is exchanged with the pair core through a
2-core HBM AllGather — every core uniformly reads back the rank-0 half
(for s=0 cores the received halo is garbage, but those j-tiles are killed
by the exp bias, keeping the program SPMD-uniform). Attention blocks run in
order a=2,3,0,1 so the halo-consuming blocks come last, hiding the
collective. Each core emits a partial o-projection [2048, 2048] in bf16;
the host sums the 4 group partials per half.

Frame indexing: j-tiles live in a 6*512-token frame [s*2048-1024, (s+1)*2048);
frame tiles 0,1 = halo (received), 2..5 = own (computed). Attention block a
covers query frame tile 2+a and j frame tiles a, a+1, a+2.

Dataflow: host pre-transposes x and all weights into partition-major
contiguous layouts so every DMA is wide. Projections and attention matmuls
run in bf16 (fp32 PSUM accumulation); rsqrt for RMSNorm is exp(-0.5*ln(x))
and the activation-table candidates are restricted so ACT stays on the
natural_log_exp table for the whole kernel (a single table load).
Attention per 512-query block in S.T orientation: S.T[j,i] = kT.T @ qT per
128-j tile, P.T = exp(S.T/16 + bias), constant triangle masks on the 8
window-edge tiles, softmax denominator via ones-matmul (no max-subtraction:
RMSNorm bounds |scores| <= 16), y.T = v.T @ P.T scaled by the reciprocal
denominator, consumed as lhsT by the o-projection.
"""

import sys

if "/opt/trn_rl_repo" not in sys.path:
    sys.path.insert(0, "/opt/trn_rl_repo")

import numpy as np

try:
    import ml_dtypes
    BF16 = ml_dtypes.bfloat16
except ImportError:
    BF16 = None

T, DM, NH, NKV, HD, WIN = 4096, 2048, 8, 4, 256, 1024
EPS, BASE = 1e-6, 10000.0
NG, NS = 4, 2
NQ = 2048
NKO = 16          # 2048 / 128 contraction subtiles
NA = 4            # 512-query attention blocks per core
NOT = 4           # own 512-token tiles per core
SCALE = 1.0 / 16.0
NEG = -1.0e5

_cache = {}


def _host_prep(x, pos, Wq, Wk, Wv, Wo, q_norm_w, k_norm_w):
    x = np.asarray(x, np.float32).reshape(T, DM)
    xT = np.ascontiguousarray(x.T)
    pos_f = np.asarray(pos).astype(np.float64)
    m = np.arange(128)
    invf = BASE ** (-m / 128.0)

    Wq = np.asarray(Wq, np.float32)
    Wk = np.asarray(Wk, np.float32)
    Wv = np.asarray(Wv, np.float32)
    Wo = np.asarray(Wo, np.float32)
    qnw = np.asarray(q_norm_w, np.float32)
    knw = np.asarray(k_norm_w, np.float32)

    ones = np.ones((128, 128), np.float32)
    r0T = np.zeros((128, 128), np.float32)
    a = np.arange(64)
    r0T[2 * a, 2 * a + 1] = 1.0
    r0T[2 * a + 1, 2 * a] = -1.0
    qw2 = np.ascontiguousarray(np.stack([qnw[:128], qnw[128:]], axis=1))
    kw2 = np.ascontiguousarray(np.stack([knw[:128], knw[128:]], axis=1))

    # masks for 512-wide attention blocks: m=0..3 far edge, m=8..11 diagonal
    jp = np.arange(128)[:, None]
    ip = np.arange(512)[None, :]
    tris = []
    for mm_ in range(4):
        tris.append(jp >= ip + 1 - 128 * mm_)         # far masks F_m
    for mm_ in range(4):
        tris.append(jp <= ip - 128 * mm_)             # diag masks D_{m+8}
    tri = np.concatenate(tris, axis=1).astype(BF16)   # [128, 8*512]

    def pack_w(WT, cols):
        # WT: [DM, cols] -> [2, 128, 8, cols] half-major so each half-load
        # is a fully contiguous per-partition run
        return np.ascontiguousarray(
            WT.reshape(2, 8, 128, cols).transpose(0, 2, 1, 3)).astype(BF16)

    in_maps = []
    for g in range(NG):
        for s in range(NS):
            xT_c = xT[:, s * 2048:(s + 1) * 2048]
            # [128, NOT*2, 8, 512]: index (own_tile*2 + half) -> 8KB/partition
            xt4 = np.ascontiguousarray(
                xT_c.reshape(2, 8, 128, NOT, 512)
                .transpose(2, 3, 0, 1, 4)
                .reshape(128, NOT * 2, 8, 512)).astype(BF16)
            p = pos_f[s * 2048:(s + 1) * 2048]
            ang = p[None, :] * invf[:, None]
            cosk = np.ascontiguousarray(np.cos(ang), dtype=np.float32)
            sink = np.ascontiguousarray(np.sin(ang), dtype=np.float32)

            kbias = np.zeros((128, 24), np.float32)
            if s == 0:
                kbias[:, :8] = NEG

            wqT = Wq[2 * g * HD:(2 * g + 2) * HD, :].T   # [DM, 512]
            wkT = Wk[g * HD:(g + 1) * HD, :].T           # [DM, 256]
            wvT = Wv[g * HD:(g + 1) * HD, :].T           # [DM, 256]
            woT = Wo[:, 2 * g * HD:(2 * g + 2) * HD].T   # [512, DM]
            wo4 = np.ascontiguousarray(
                woT.reshape(4, 128, DM).transpose(1, 0, 2)).astype(BF16)

            in_maps.append({
                "xt4": xt4,
                "cosk": cosk,
                "sink": sink,
                "wqT": pack_w(wqT, 512),
                "wkT": pack_w(wkT, 256),
                "wvT": pack_w(wvT, 256),
                "woT": wo4,
                "ones_bf": ones.astype(BF16),
                "r0T": r0T.astype(BF16),
                "qw": qw2,
                "kw": kw2,
                "kbias": kbias,
                "tri": tri,
            })
    return in_maps


def _build_program():
    if "nc" in _cache:
        return _cache["nc"]

    import concourse.bass as bass
    import concourse.mybir as mybir
    import concourse.tile as tile
    from concourse import bacc
    from contextlib import ExitStack

    f32 = mybir.dt.float32
    bf16 = mybir.dt.bfloat16
    AF = mybir.ActivationFunctionType
    OP = mybir.AluOpType

    nc = bacc.Bacc("TRN2", target_bir_lowering=False, debug=False,
                   enable_asserts=False, num_devices=8)

    # All ACT functions used here (Exp, Ln, Square, Copy) are members of the
    # natural_log_exp_and_others table set, but the placement pass assigns
    # each function its first matching set, which thrashes ~2.7us table
    # reloads between rmsnorm and attention. Restrict the candidates this
    # instance's pass sees (positions preserved -> set ids stay valid) so a
    # single table load serves the whole kernel.
    import types as _types
    from concourse.hw_specs import get_activation_tables as _gat
    import bass_rust as _bass_rust

    def _pinned_act_table_loads(self):
        has_activation = any(
            isinstance(i, mybir.InstActivation)
            for b in self.main_func.blocks
            for i in b.instructions
        )
        if not has_activation:
            return
        AF_ = mybir.ActivationFunctionType
        ours = {AF_.Exp, AF_.Ln, AF_.Square, AF_.Copy}
        tables = []
        for name, fns in _gat(self.m.arch).items():
            if name != "natural_log_exp_and_others":
                fns = fns - ours
            tables.append((name, fns))
        _bass_rust.insert_act_table_loads(self, tables)

    nc.insert_act_table_loads = _types.MethodType(_pinned_act_table_loads, nc)

    xt4_d = nc.dram_tensor("xt4", [128, NOT * 2, 8, 512], bf16, kind="ExternalInput")
    cosk_d = nc.dram_tensor("cosk", [128, 2048], f32, kind="ExternalInput")
    sink_d = nc.dram_tensor("sink", [128, 2048], f32, kind="ExternalInput")
    wq_d = nc.dram_tensor("wqT", [2, 128, 8, 512], bf16, kind="ExternalInput")
    wk_d = nc.dram_tensor("wkT", [2, 128, 8, 256], bf16, kind="ExternalInput")
    wv_d = nc.dram_tensor("wvT", [2, 128, 8, 256], bf16, kind="ExternalInput")
    wo_d = nc.dram_tensor("woT", [128, 4, DM], bf16, kind="ExternalInput")
    onesbf_d = nc.dram_tensor("ones_bf", [128, 128], bf16, kind="ExternalInput")
    r0_d = nc.dram_tensor("r0T", [128, 128], bf16, kind="ExternalInput")
    qw_d = nc.dram_tensor("qw", [128, 2], f32, kind="ExternalInput")
    kw_d = nc.dram_tensor("kw", [128, 2], f32, kind="ExternalInput")
    kb_d = nc.dram_tensor("kbias", [128, 24], f32, kind="ExternalInput")
    tri_d = nc.dram_tensor("tri", [128, 8 * 512], bf16, kind="ExternalInput")
    o_d = nc.dram_tensor("o_part", [NQ, DM], bf16, kind="ExternalOutput")

    PAIRS = [[0, 1], [2, 3], [4, 5], [6, 7]]

    with tile.TileContext(nc) as tc, ExitStack() as ctx:
        cpool = ctx.enter_context(tc.tile_pool(name="consts", bufs=1))
        xpool = ctx.enter_context(tc.tile_pool(name="xt", bufs=4))
        tabpool = ctx.enter_context(tc.tile_pool(name="tab", bufs=4))
        kpool = ctx.enter_context(tc.tile_pool(name="kring", bufs=6))
        vpool = ctx.enter_context(tc.tile_pool(name="vring", bufs=6))
        scpool = ctx.enter_context(tc.tile_pool(name="scratch", bufs=3))
        spool = ctx.enter_context(tc.tile_pool(name="small", bufs=2))
        qpool = ctx.enter_context(tc.tile_pool(name="qt", bufs=4))
        ptpool = ctx.enter_context(tc.tile_pool(name="pt", bufs=6))
        ypool = ctx.enter_context(tc.tile_pool(name="yt", bufs=3))
        opool = ctx.enter_context(tc.tile_pool(name="osb", bufs=4))
        dpool = ctx.enter_context(tc.tile_pool(name="dram", bufs=1, space="DRAM"))
        pp_proj = ctx.enter_context(tc.tile_pool(name="pproj", bufs=3, space="PSUM"))
        pp_small = ctx.enter_context(tc.tile_pool(name="psmall", bufs=2, space="PSUM"))
        pp_acc = ctx.enter_context(tc.tile_pool(name="pacc", bufs=3, space="PSUM"))

        # ---- resident constants / weights, ordered for startup latency ----
        wk_sb = cpool.tile([128, NKO, 256], bf16, tag="wk")
        nc.sync.dma_start(wk_sb[:, 0:4, :], wk_d.ap()[0, :, 0:4, :])
        xts = {}

        def load_x(o, split=False):
            xth = []
            for half in range(2):
                xt = xpool.tile([128, 8, 512], bf16, tag="xt")
                if split and half == 0:
                    nc.sync.dma_start(xt[:, 0:4, :], xt4_d.ap()[:, o * 2, 0:4, :])
                    nc.sync.dma_start(xt[:, 4:8, :], xt4_d.ap()[:, o * 2, 4:8, :])
                else:
                    nc.sync.dma_start(xt[:], xt4_d.ap()[:, o * 2 + half, :, :])
                xth.append(xt)
            xts[o] = xth

        load_x(2, split=True)
        nc.sync.dma_start(wk_sb[:, 4:8, :], wk_d.ap()[0, :, 4:8, :])
        nc.sync.dma_start(wk_sb[:, 8:16, :], wk_d.ap()[1])
        wv_sb = cpool.tile([128, NKO, 256], bf16, tag="wv")
        nc.sync.dma_start(wv_sb[:, 0:8, :], wv_d.ap()[0])
        nc.sync.dma_start(wv_sb[:, 8:16, :], wv_d.ap()[1])
        ones_sb = cpool.tile([128, 128], bf16, tag="ones")
        nc.sync.dma_start(ones_sb[:], onesbf_d.ap())
        onesbf_sb = ones_sb
        r0_sb = cpool.tile([128, 128], bf16, tag="r0")
        nc.sync.dma_start(r0_sb[:], r0_d.ap())
        qw_sb = cpool.tile([128, 2], f32, tag="qwt")
        nc.sync.dma_start(qw_sb[:], qw_d.ap())
        kw_sb = cpool.tile([128, 2], f32, tag="kwt")
        nc.sync.dma_start(kw_sb[:], kw_d.ap())
        kb_sb = cpool.tile([128, 24], f32, tag="kb")
        nc.sync.dma_start(kb_sb[:], kb_d.ap())
        eps_sb = cpool.tile([128, 1], f32, tag="eps")
        nc.vector.memset(eps_sb[:], EPS)
        zero_sb = cpool.tile([128, 1], f32, tag="zero")
        nc.vector.memset(zero_sb[:], 0.0)
        cos_tiles = {}
        sin_tiles = {}

        def load_tab(o):
            ct_ = tabpool.tile([128, 512], f32, tag="cos")
            nc.sync.dma_start(ct_[:], cosk_d.ap()[:, o * 512:(o + 1) * 512])
            st_ = tabpool.tile([128, 512], f32, tag="sin")
            nc.sync.dma_start(st_[:], sink_d.ap()[:, o * 512:(o + 1) * 512])
            cos_tiles[o] = ct_
            sin_tiles[o] = st_

        load_tab(2)
        wq_sb = cpool.tile([128, NKO, 512], bf16, tag="wq")
        nc.sync.dma_start(wq_sb[:, 0:8, :], wq_d.ap()[0])
        nc.sync.dma_start(wq_sb[:, 8:16, :], wq_d.ap()[1])
        # allocated now, loaded later (first used by attention/o blocks)
        tri_sb = cpool.tile([128, 8 * 512], bf16, tag="tri")
        wo_sb = cpool.tile([128, 4, DM], bf16, tag="wo")

        # frame tiles: 0,1 halo (exchanged), 2..5 own (computed)
        kt_tiles = [None] * 6
        vt_tiles = [None] * 6
        qt_tiles = [None] * 4

        def norm_rope(src_ps, w_sb, cos_t, sin_t, dst, dsti):
            """src_ps: two PSUM [128, 512] tiles (one head's 2 d-subtiles),
            transposed projection over 512 tokens. Writes RMSNorm+RoPE (bf16)
            into dst[:, dsti+u, :]."""
            z2 = scpool.tile([128, 2, 512], bf16, tag="z2")
            for u in range(2):
                nc.scalar.activation(z2[:, u, :], src_ps[u][:], AF.Square,
                                     bias=zero_sb[:])
            ssq = pp_small.tile([128, 512], f32, tag="psm")
            for u in range(2):
                nc.tensor.matmul(ssq[:], ones_sb[:], z2[:, u, :],
                                 start=(u == 0), stop=(u == 1))
            # rs = (ssq/HD + eps) ** -0.5 via ln+exp (one ACT table set)
            lt = spool.tile([128, 512], f32, tag="sq")
            nc.scalar.activation(lt[:], ssq[:], AF.Ln, bias=eps_sb[:],
                                 scale=1.0 / HD)
            rs = spool.tile([128, 512], f32, tag="rs")
            nc.scalar.activation(rs[:], lt[:], AF.Exp, bias=zero_sb[:],
                                 scale=-0.5)
            znw = scpool.tile([128, 2, 512], bf16, tag="znw")
            t1 = scpool.tile([128, 2, 512], bf16, tag="t1")
            for u in range(2):
                nc.vector.scalar_tensor_tensor(
                    znw[:, u, :], src_ps[u][:], w_sb[:, u:u + 1], rs[:],
                    OP.mult, OP.mult)
                rot = pp_small.tile([128, 512], f32, tag="psm")
                nc.tensor.matmul(rot[:], r0_sb[:], znw[:, u, :], start=True, stop=True)
                nc.vector.tensor_tensor(t1[:, u, :], znw[:, u, :], cos_t, OP.mult)
                tmp = spool.tile([128, 512], f32, tag="tmp")
                nc.vector.tensor_tensor(tmp[:], rot[:], sin_t, OP.mult)
                nc.vector.tensor_tensor(dst[:, dsti + u, :], t1[:, u, :], tmp[:], OP.add)

        def proj_kv(o):
            """K/V projection + norm/rope for own tile o (frame tile 2+o)."""
            xth = xts.pop(o)
            cos_t = cos_tiles[o]
            sin_t = sin_tiles[o]

            k0_ps = pp_proj.tile([128, 512], f32, tag="pj")
            k1_ps = pp_proj.tile([128, 512], f32, tag="pj")
            k_ps = [k0_ps, k1_ps]
            for dsub in range(2):
                for ko in range(NKO):
                    nc.tensor.matmul(k_ps[dsub][:],
                                     wk_sb[:, ko, dsub * 128:(dsub + 1) * 128],
                                     xth[ko // 8][:, ko % 8, :],
                                     start=(ko == 0), stop=(ko == NKO - 1))
            kt = kpool.tile([128, 2, 512], bf16, tag="kt")
            norm_rope(k_ps, kw_sb, cos_t[:], sin_t[:], kt, 0)
            kt_tiles[2 + o] = kt

            vt = vpool.tile([128, 4, 256], bf16, tag="vt")
            for vh in range(2):
                v_ps = pp_proj.tile([128, 2, 256], f32, tag="pj")
                for ms in range(2):
                    msub = vh * 2 + ms
                    for ko in range(NKO):
                        nc.tensor.matmul(v_ps[:, ms, :],
                                         xth[ko // 8][:, ko % 8, msub * 128:(msub + 1) * 128],
                                         wv_sb[:, ko, :],
                                         start=(ko == 0), stop=(ko == NKO - 1))
                for ms in range(2):
                    nc.vector.tensor_copy(vt[:, vh * 2 + ms, :], v_ps[:, ms, :])
            vt_tiles[2 + o] = vt

            # q projections (2 heads, N=512) for query tile o
            qt_sb = qpool.tile([128, 4, 512], bf16, tag="q")
            for h in range(2):
                q0_ps = pp_proj.tile([128, 512], f32, tag="pj")
                q1_ps = pp_proj.tile([128, 512], f32, tag="pj")
                q_ps = [q0_ps, q1_ps]
                for u in range(2):
                    dsub = 2 * h + u
                    for ko in range(NKO):
                        nc.tensor.matmul(q_ps[u][:],
                                         wq_sb[:, ko, dsub * 128:(dsub + 1) * 128],
                                         xth[ko // 8][:, ko % 8, :],
                                         start=(ko == 0), stop=(ko == NKO - 1))
                norm_rope(q_ps, qw_sb, cos_t[:], sin_t[:], qt_sb, 2 * h)
            qt_tiles[o] = qt_sb

        # ---- projection phase: own tiles 2,3 first (the pair's halo), then
        # the halo exchange, then own tiles 0,1 ----
        proj_kv(2)
        load_x(3)
        load_tab(3)
        proj_kv(3)

        # halo exchange: my frame tiles 4,5 (own 2,3) -> pair's frame 0,1.
        # Both pair members contribute; everyone reads back the rank-0 half
        # (masked garbage on s=0 cores).
        # The whole exchange lives on the otherwise-idle gpsimd queue so its
        # waits never head-of-line-block the sync queue's DMA stream.
        send_b = dpool.tile([128, 4096], bf16, tag="sendb")
        nc.gpsimd.dma_start(send_b[:, 0:1024],
                            kt_tiles[4][:].rearrange("p a b -> p (a b)"))
        nc.gpsimd.dma_start(send_b[:, 1024:2048],
                            kt_tiles[5][:].rearrange("p a b -> p (a b)"))
        nc.gpsimd.dma_start(send_b[:, 2048:3072],
                            vt_tiles[4][:].rearrange("p a b -> p (a b)"))
        nc.gpsimd.dma_start(send_b[:, 3072:4096],
                            vt_tiles[5][:].rearrange("p a b -> p (a b)"))
        recv_b = dpool.tile([2, 128, 4096], bf16, tag="recvb")
        nc.gpsimd.collective_compute(
            "AllGather",
            mybir.AluOpType.bypass,
            replica_groups=PAIRS,
            ins=[send_b.opt()],
            outs=[recv_b.opt()],
        )

        load_x(0)
        load_tab(0)
        proj_kv(0)
        load_x(1)
        load_tab(1)
        nc.sync.dma_start(tri_sb[:], tri_d.ap())
        nc.sync.dma_start(wo_sb[:], wo_d.ap())
        proj_kv(1)

        def recv_halo():
            for f in range(2):
                kt = kpool.tile([128, 2, 512], bf16, tag="kt")
                nc.gpsimd.dma_start(kt[:].rearrange("p a b -> p (a b)"),
                                    recv_b[0, :, f * 1024:(f + 1) * 1024])
                kt_tiles[f] = kt
                vt = vpool.tile([128, 4, 256], bf16, tag="vt")
                nc.gpsimd.dma_start(vt[:].rearrange("p a b -> p (a b)"),
                                    recv_b[0, :, 2048 + f * 1024:2048 + (f + 1) * 1024])
                vt_tiles[f] = vt

        # ---- attention phase: halo-free blocks first ----
        for a in (2, 3, 0, 1):
            if a == 0:
                recv_halo()
            qt_sb = qt_tiles[a]

            # attention for 512-query block a
            yt_sb = ypool.tile([128, 4, 512], bf16, tag="y")
            for h in range(2):
                dn_ps = pp_acc.tile([128, 512], f32, tag="pac")
                y0_ps = pp_acc.tile([128, 512], f32, tag="pac")
                y1_ps = pp_acc.tile([128, 512], f32, tag="pac")
                y_ps = [y0_ps, y1_ps]
                for mi, mrel in enumerate([3, 0, 1, 2] + list(range(4, 12))):
                    jt = 4 * a + mrel
                    ct, jh = jt // 4, jt % 4
                    ktc = kt_tiles[ct]
                    vtc = vt_tiles[ct]
                    # active query range: edge tiles are mostly masked
                    if mrel <= 2:
                        ia, ib = 0, 128 * (mrel + 1)
                    elif mrel >= 9:
                        ia, ib = 128 * (mrel - 8), 512
                    else:
                        ia, ib = 0, 512
                    pt = ptpool.tile([128, 512], bf16, tag="p")
                    st = pp_small.tile([128, 512], f32, tag="psm")
                    for u in range(2):
                        nc.tensor.matmul(st[:, ia:ib],
                                         ktc[:, u, jh * 128:(jh + 1) * 128],
                                         qt_sb[:, 2 * h + u, ia:ib],
                                         start=(u == 0), stop=(u == 1))
                    nc.scalar.activation(pt[:, ia:ib], st[:, ia:ib], AF.Exp,
                                         bias=kb_sb[:, jt:jt + 1], scale=SCALE)
                    if mrel < 4:
                        nc.vector.tensor_tensor(
                            pt[:, ia:ib], pt[:, ia:ib],
                            tri_sb[:, mrel * 512 + ia:mrel * 512 + ib], OP.mult)
                    elif mrel >= 8:
                        nc.vector.tensor_tensor(
                            pt[:, ia:ib], pt[:, ia:ib],
                            tri_sb[:, (mrel - 4) * 512 + ia:(mrel - 4) * 512 + ib],
                            OP.mult)
                    first, last = (mi == 0), (mrel == 11)
                    nc.tensor.matmul(dn_ps[:, ia:ib], onesbf_sb[:], pt[:, ia:ib],
                                     start=first, stop=last, skip_group_check=True)
                    for dh in range(2):
                        nc.tensor.matmul(y_ps[dh][:, ia:ib],
                                         vtc[:, jh, dh * 128:(dh + 1) * 128],
                                         pt[:, ia:ib], start=first, stop=last,
                                         skip_group_check=True)
                rc = spool.tile([128, 512], f32, tag="rc")
                nc.vector.reciprocal_approx_fast(rc[:], dn_ps[:])
                for msub in range(4):
                    c0, c1 = msub * 128, (msub + 1) * 128
                    for dh in range(2):
                        nc.vector.tensor_tensor(yt_sb[:, 2 * h + dh, c0:c1],
                                                y_ps[dh][:, c0:c1], rc[:, c0:c1],
                                                OP.mult)

            # partial o-projection for the 512-query block
            for msub in range(4):
                for dmh in range(2):
                    o_sb = opool.tile([128, 1024], bf16, tag="o")
                    for dq in range(2):
                        c0 = (dmh * 2 + dq) * 512
                        o_ps = pp_proj.tile([128, 512], f32, tag="pj")
                        for hd in range(4):
                            nc.tensor.matmul(o_ps[:],
                                             yt_sb[:, hd, msub * 128:(msub + 1) * 128],
                                             wo_sb[:, hd, c0:c0 + 512],
                                             start=(hd == 0), stop=(hd == 3))
                        if dq == 0:
                            nc.scalar.copy(o_sb[:, dq * 512:(dq + 1) * 512], o_ps[:])
                        else:
                            nc.vector.tensor_copy(o_sb[:, dq * 512:(dq + 1) * 512], o_ps[:])
                    r0_ = a * 512 + msub * 128
                    nc.sync.dma_start(o_d.ap()[r0_:r0_ + 128, dmh * 1024:(dmh + 1) * 1024],
                                      o_sb[:])

    nc.compile()
    _cache["nc"] = nc
    return nc


def _run(inputs, trace=False):
    from concourse.bass_utils import run_bass_kernel_spmd

    nc = _build_program()
    in_maps = _host_prep(**inputs)
    res = run_bass_kernel_spmd(nc, in_maps, core_ids=list(range(8)), trace=trace)
    full = np.zeros((T, DM), np.float32)
    for g in range(NG):
        for s in range(NS):
            full[s * 2048:(s + 1) * 2048] += np.asarray(
                res.results[g * 2 + s]["o_part"], dtype=np.float32)
    return full.reshape(1, T, DM), res


def kernel(**inputs):
    return _run(inputs, trace=False)[0]
